# revision 7
# baseline (speedup 1.0000x reference)
"""Trainium2 Bass kernel for nn_Net_12481174962824 (binarized CNN) — v2.

Data-parallel over 8 cores (512 images each). Within a core, images are
processed in chunks of 64, split into two 32-image sets living on partition
halves (set A -> partitions 0-63, set B -> 64-127) so every post-conv
pool/quant chain runs at full 128-partition width with no cross-partition
copies.

  conv1: bf16 two-split (hi/mid) im2col matmul, K=20 per 32-row group
    (2 splits x 9 taps + 2 ones-rows carrying the bias 3*b1 as hi/lo bf16
    weights; the x3 activation scale is folded into the +/-3 weights).
    4 row-groups x 2 col-groups = 8-way PE tiling, PSUM [128, 14, 28].
  pool+quant: banks alternate between a DVE pooled-reduce path and an
    ACT-drain + DVE tensor_max path to balance the two PSUM-reader engines;
    round via min(3,x)+2^23 (RNE) then ACT Relu(x-C) -> fp8.
  conv2/conv3: fp8e4 DoubleRow matmuls — activations {0..3} and +/-1
    weights are exact in fp8; each MM's in-cell pair covers taps (dy,dy+1)
    via the 16-byte row pitch, so 3 taps/column need 2 MMs per dx (6 total
    per 9-tap conv). Sets run as separate row/col tile groups.
  fc1: fp8 DoubleRow over 14 feature-chunks (zero-padded), fc2 fp32 with
    fc1 activations stationary so logits land batch-on-partitions.
"""

import numpy as np
import ml_dtypes

BF16 = ml_dtypes.bfloat16
FP8 = ml_dtypes.float8_e4m3
F32 = np.float32
C_RND = np.float32(12582912.0)  # 1.5 * 2**23: (x + C) - C rounds to int (RNE)
N_CORES = 8
B_CORE = 512
NB = 64               # images per chunk
NCHUNK = B_CORE // NB
NBG = NB // 4         # images per conv1 row-group (16)


def _f32(x):
    return np.asarray(x, dtype=np.float32)


def _prep(w1, b1, w2, g1, be1, m1, v1, w3, g2, be2, m2, v2, fw1, fb1, fw2, fb2):
    """Host prep of small weight tensors. Returns dict of np arrays."""
    sg = lambda w: np.where(_f32(w) >= 0, np.float32(1), np.float32(-1))

    # conv1 lhsT [128, 64]: rows 32g + 9s + t = 3*sign(w1); rows 32g+18/19 = bias 3*b1 hi/lo
    w1b = sg(w1)  # [64,1,3,3]
    base = (3.0 * w1b[:, 0].reshape(64, 9).T).astype(F32)  # [9 taps, 64]
    b3 = (3.0 * _f32(b1)).astype(F32)
    b3hi = b3.astype(BF16)
    b3lo = (b3 - b3hi.astype(F32)).astype(BF16)
    w1l = np.zeros((128, 64), dtype=BF16)
    for g in range(4):
        for s in range(3):
            w1l[32 * g + 9 * s: 32 * g + 9 * s + 9, :] = base.astype(BF16)
        w1l[32 * g + 27, :] = b3hi
        w1l[32 * g + 28, :] = b3lo

    # conv2/conv3 DoubleRow weights: [128, 3(dx), 2(j), 64] with rows 64-127 duplicated
    def dr_pair(w):
        wb = sg(w)  # [cout, cin, dy, dx]
        a = np.zeros((64, 3, 2, 64), dtype=FP8)
        b = np.zeros((64, 3, 2, 64), dtype=FP8)
        for dx in range(3):
            a[:, dx, 0, :] = wb[:, :, 0, dx].T.astype(FP8)
            a[:, dx, 1, :] = wb[:, :, 1, dx].T.astype(FP8)
            b[:, dx, 0, :] = wb[:, :, 2, dx].T.astype(FP8)
        a2 = np.concatenate([a, a], axis=0).reshape(128, 3 * 2 * 64)
        b2 = np.concatenate([b, b], axis=0).reshape(128, 3 * 2 * 64)
        return a2, b2

    w2l, w2l2 = dr_pair(w2)
    w3l, w3l2 = dr_pair(w3)

    # BN folds (fp32 arithmetic like the reference); activations carry a 3x scale
    def fold(g, be, m, v):
        rs = (np.float32(1.0) / np.sqrt(_f32(v) + np.float32(1e-4))).astype(F32)
        inv = (_f32(g) * rs).astype(F32)
        assert (inv > 0).all(), "negative BN scale: pool/quant commute breaks"
        s = inv.astype(F32)                                   # 3*c*inv, c=1/3
        b = (np.float32(3.0) * (_f32(be) - _f32(m) * inv)).astype(F32)
        return s, b

    s1v, b1v = fold(g1, be1, m1, v1)
    s2v, b2v = fold(g2, be2, m2, v2)
    cvec = np.zeros((128, 6), dtype=F32)
    for half in range(2):
        cvec[64 * half:64 * half + 64, 0] = s1v
        cvec[64 * half:64 * half + 64, 1] = b1v
        cvec[64 * half:64 * half + 64, 2] = s2v
        cvec[64 * half:64 * half + 64, 3] = b2v
    cvec[:, 4] = -C_RND

    # fc1 DoubleRow lhsT [128, 4(m), 7(u), 2(j), 128]:
    # partition p = 64*jh + ch holds feature (4u+2j+jh)*64 + ch
    fw1b = sg(fw1)  # [512, 1600]
    fw1l = np.zeros((128, 4, 7, 2, 128), dtype=FP8)
    for m in range(4):
        for u in range(7):
            for j in range(2):
                for jh in range(2):
                    p = 4 * u + 2 * j + jh
                    if p >= 25:
                        continue
                    fw1l[64 * jh:64 * jh + 64, m, u, j, :] = (
                        fw1b[128 * m:128 * (m + 1), 64 * p:64 * (p + 1)].T.astype(FP8))
    fw1l = fw1l.reshape(128, 4 * 7 * 2 * 128)

    # fc2 lhsT [128, 4, 10] fp32: row j of chunk k2 = fc1-feature 128*k2+j
    fw2l = np.zeros((128, 4, 10), dtype=F32)
    for k2 in range(4):
        fw2l[:, k2, :] = _f32(fw2)[:, 128 * k2:128 * (k2 + 1)].T
    fw2l = fw2l.reshape(128, 40)

    fb1v = _f32(fb1).reshape(4, 128).T.copy()  # [128, 4]
    fb2v = _f32(fb2).reshape(10, 1).copy()     # [10, 1]
    onesv = np.ones(2 * 16 * 840, dtype=BF16)
    return dict(w1l=w1l, w2l=w2l, w2l2=w2l2, w3l=w3l, w3l2=w3l2, cvec=cvec,
                fw1l=fw1l, fw2l=fw2l, fb1v=fb1v, fb2v=fb2v, onesv=onesv)


def _split_x(x_shard):
    """[512,28,28] f32 -> three padded bf16 split tensors [512*900+64]."""
    S = np.zeros((B_CORE, 30, 30), dtype=F32)
    S[:, 1:29, 1:29] = x_shard
    S = S.reshape(-1)
    hi = S.astype(BF16)
    r = (S - hi.astype(F32)).astype(F32)
    mid = r.astype(BF16)
    lo = (r - mid.astype(F32)).astype(BF16)
    pad = np.zeros(64, dtype=BF16)
    return (np.concatenate([hi, pad]), np.concatenate([mid, pad]),
            np.concatenate([lo, pad]))


def _build_nc():
    import concourse.bass as bass
    import concourse.bacc as bacc
    import concourse.tile as tile
    import concourse.mybir as mybir
    from contextlib import ExitStack

    fp32 = mybir.dt.float32
    bf16 = mybir.dt.bfloat16
    f16 = mybir.dt.float16
    f8 = mybir.dt.float8e4
    AX = mybir.AxisListType.X
    AF = mybir.ActivationFunctionType
    ALU = mybir.AluOpType
    DR = mybir.MatmulPerfMode.DoubleRow
    c13 = float(np.float32(1.0) / np.float32(3.0))

    nc = bacc.Bacc("TRN2", target_bir_lowering=False)
    d_shi = nc.dram_tensor("s_hi", [B_CORE * 900 + 64], bf16, kind="ExternalInput")
    d_smid = nc.dram_tensor("s_mid", [B_CORE * 900 + 64], bf16, kind="ExternalInput")
    d_slo = nc.dram_tensor("s_lo", [B_CORE * 900 + 64], bf16, kind="ExternalInput")
    d_w1 = nc.dram_tensor("w1l", [128, 64], bf16, kind="ExternalInput")
    d_w2 = nc.dram_tensor("w2l", [128, 384], f8, kind="ExternalInput")
    d_w2b = nc.dram_tensor("w2l2", [128, 384], f8, kind="ExternalInput")
    d_w3 = nc.dram_tensor("w3l", [128, 384], f8, kind="ExternalInput")
    d_w3b = nc.dram_tensor("w3l2", [128, 384], f8, kind="ExternalInput")
    d_cv = nc.dram_tensor("cvec", [128, 6], fp32, kind="ExternalInput")
    d_fw1 = nc.dram_tensor("fw1l", [128, 4 * 7 * 2 * 128], f8, kind="ExternalInput")
    d_fw2 = nc.dram_tensor("fw2l", [128, 40], fp32, kind="ExternalInput")
    d_fb1 = nc.dram_tensor("fb1v", [128, 4], fp32, kind="ExternalInput")
    d_fb2 = nc.dram_tensor("fb2v", [10, 1], fp32, kind="ExternalInput")
    d_ones = nc.dram_tensor("onesv", [2 * 16 * 840], bf16, kind="ExternalInput")
    d_out = nc.dram_tensor("out", [B_CORE, 10], fp32, kind="ExternalOutput")

    splits = [d_shi, d_smid, d_slo]

    with tile.TileContext(nc) as tc, ExitStack() as ctx:
        sg = ctx.enter_context(tc.tile_pool(name="sg", bufs=1))
        tmp = ctx.enter_context(tc.tile_pool(name="tmp", bufs=6))
        psA = ctx.enter_context(tc.tile_pool(name="psA", bufs=3, space="PSUM"))
        psB = ctx.enter_context(tc.tile_pool(name="psB", bufs=3, space="PSUM"))
        psC = ctx.enter_context(tc.tile_pool(name="psC", bufs=2, space="PSUM"))

        # --- weights / constants ---
        W1 = sg.tile([128, 64], bf16)
        nc.sync.dma_start(out=W1, in_=d_w1[:, :])
        W2 = sg.tile([128, 3, 2, 64], f8)
        nc.sync.dma_start(out=W2, in_=d_w2[:, :].rearrange("p (d j c) -> p d j c", d=3, j=2))
        W2b = sg.tile([128, 3, 2, 64], f8)
        nc.sync.dma_start(out=W2b, in_=d_w2b[:, :].rearrange("p (d j c) -> p d j c", d=3, j=2))
        W3 = sg.tile([128, 3, 2, 64], f8)
        nc.sync.dma_start(out=W3, in_=d_w3[:, :].rearrange("p (d j c) -> p d j c", d=3, j=2))
        W3b = sg.tile([128, 3, 2, 64], f8)
        nc.sync.dma_start(out=W3b, in_=d_w3b[:, :].rearrange("p (d j c) -> p d j c", d=3, j=2))
        CV = sg.tile([128, 6], fp32)
        nc.sync.dma_start(out=CV, in_=d_cv[:, :])
        FW1 = sg.tile([128, 4, 7, 2, 128], f8)
        nc.sync.dma_start(out=FW1, in_=d_fw1[:, :].rearrange("p (m u j c) -> p m u j c", m=4, u=7, j=2))
        FW2 = sg.tile([128, 4, 10], fp32)
        nc.sync.dma_start(out=FW2, in_=d_fw2[:, :].rearrange("p (k m) -> p k m", k=4))
        FB1 = sg.tile([128, 4], fp32)
        nc.sync.dma_start(out=FB1, in_=d_fb1[:, :])
        FB2T = sg.tile([128, 10], fp32)
        fb2b = bass.AP(tensor=d_fb2[:, :].tensor, offset=0, ap=[[0, 128], [1, 10]])
        nc.sync.dma_start(out=FB2T, in_=fb2b)

        # --- persistent ping-pong activation tiles ---
        T1s = [sg.tile([128, 16, 840], bf16, name=f"T1_{i}") for i in range(2)]
        T2s = [sg.tile([128, 33, 16, 16], f8, name=f"T2_{i}") for i in range(2)]
        T3s = [sg.tile([128, 33, 8, 16], f8, name=f"T3_{i}") for i in range(2)]
        Q3s = [sg.tile([128, 25, 32], f8, name=f"Q3_{i}") for i in range(2)]
        F2 = sg.tile([128, 14, 512], f8)
        H1 = [None] * 4

        # one-time pads/ones
        for i in range(2):
            T2, T3 = T2s[i], T3s[i]
            nc.gpsimd.memset(T2[:, :, 0, :], 0)
            nc.gpsimd.memset(T2[:, :, 15, :], 0)
            nc.gpsimd.memset(T2[:, :, 1:15, 0], 0)
            nc.gpsimd.memset(T2[:, :, 1:15, 15], 0)
            nc.gpsimd.memset(T2[:, 32, 1:15, 1:15], 0)
            nc.gpsimd.memset(T3[:, :, 7, :], 0)
            nc.gpsimd.memset(T3[:, :, 0:7, 7:16], 0)
            nc.gpsimd.memset(T3[:, 32, :, :], 0)
            for g in range(4):
                nc.sync.dma_start(
                    out=T1s[i][32 * g + 27:32 * g + 29, :, :].rearrange("p b x -> p (b x)"),
                    in_=bass.AP(tensor=d_ones[:].tensor, offset=0,
                                ap=[[13440, 2], [1, 13440]]))
        nc.vector.memset(F2[64:128, 12, :], 0)
        nc.vector.memset(F2[:, 13, :], 0)
        # touch Exp/Ln act tables now so the loads don't stall the softmax tail
        warm = sg.tile([128, 2], fp32, name="warm")
        nc.gpsimd.memset(warm, 1.0)
        nc.scalar.activation(out=warm[:, 0:1], in_=warm[:, 0:1], func=AF.Exp)
        nc.scalar.activation(out=warm[:, 1:2], in_=warm[:, 1:2], func=AF.Ln)

        pending_fc = []
        for ch in range(NCHUNK):
            img0 = ch * NB
            T1, T2, T3, Q3 = T1s[ch % 2], T2s[ch % 2], T3s[ch % 2], Q3s[ch % 2]

            # ---- conv1 im2col DMAs: one per (group, split, dy) ----
            for g in range(4):
                for s in range(3):
                    for dy in range(3):
                        src = bass.AP(
                            tensor=splits[s][:].tensor,
                            offset=(img0 + 16 * g) * 900 + 30 * dy,
                            ap=[[1, 3], [900, 16], [1, 840]])
                        r0 = 32 * g + 9 * s + 3 * dy
                        # lo split goes via the Pool engine's SWDGE queue so
                        # the SP/HWDGE path only carries 24 DMAs per chunk
                        if s == 2:
                            nc.gpsimd.dma_start(out=T1[r0:r0 + 3, :, :], in_=src)
                        else:
                            nc.sync.dma_start(out=T1[r0:r0 + 3, :, :], in_=src)
            while pending_fc:
                pending_fc.pop(0)()

            T1v = T1.rearrange("p b (h y x) -> p b h y x", h=2, y=14, x=30)

            # ---- conv1 + pool + quant ----
            for bb in range(16):
                for half in range(2):
                    ys = 1 + 7 * half
                    P0 = psA.tile([128, 14, 28], fp32, tag="c1", name="P0")
                    P1 = psA.tile([128, 14, 28], fp32, tag="c1", name="P1")
                    for g in range(4):
                        P = (P0, P1)[g % 2]
                        colg = 64 * (g // 2)
                        nc.tensor.matmul(
                            P[colg:colg + 64, :, :],
                            W1[32 * g:32 * g + 29, :],
                            T1v[32 * g:32 * g + 29, bb, half, :, 0:28],
                            start=True, stop=True, skip_group_check=True,
                            tile_position=(32 * g, colg))
                    R2P = tmp.tile([128, 2, 7, 14], fp32, tag="r2p", name="R2P")
                    if bb % 2 != 0:
                        # path R: DVE pooled-reduce straight from PSUM
                        for pi, P in enumerate((P0, P1)):
                            psv = P.rearrange(
                                "p (y2 dy) (x2 dx) -> p y2 x2 dy dx",
                                dy=2, dx=2)
                            nc.vector.tensor_reduce(out=R2P[:, pi, :, :],
                                                    in_=psv,
                                                    axis=mybir.AxisListType.XY,
                                                    op=ALU.max)
                    else:
                        # path D: ACT drains PSUM so DVE only does the
                        # cheap SBUF max pair + round (drain-load balance)
                        for pi, P in enumerate((P0, P1)):
                            D1 = tmp.tile([128, 14, 28], fp32, tag="d1", name="D1")
                            nc.scalar.activation(out=D1, in_=P, func=AF.Copy,
                                                 bias=0.0, scale=1.0)
                            d1v = D1.rearrange("p (y2 dy) x -> p y2 dy x", dy=2)
                            D2 = tmp.tile([128, 7, 28], fp32, tag="d2", name="D2")
                            nc.vector.tensor_max(D2, d1v[:, :, 0, :],
                                                 d1v[:, :, 1, :])
                            d2v = D2.rearrange("p y (x2 dx) -> p y x2 dx", dx=2)
                            nc.vector.tensor_max(R2P[:, pi, :, :],
                                                 d2v[:, :, :, 0],
                                                 d2v[:, :, :, 1])
                    # shared round + single paired write into slots bb, bb+16
                    R3P = tmp.tile([128, 2, 7, 14], fp32, tag="r3p", name="R3P")
                    nc.vector.tensor_scalar(out=R3P, in0=R2P, scalar1=3.0,
                                            scalar2=float(C_RND),
                                            op0=ALU.min, op1=ALU.add)
                    T2p = T2[:, 0:32, :, :].rearrange(
                        "p (b2 sl) y x -> p sl b2 y x", b2=2)
                    nc.scalar.activation(out=T2p[:, bb, :, ys:ys + 7, 1:15],
                                         in_=R3P, func=AF.Relu,
                                         bias=CV[:, 4:5], scale=1.0)

            # ---- conv2 (fp8 DoubleRow, 6 MMs per set, per-set dst banks) ----
            pitch2 = 33 * 256
            for jp in range(16):
                for si in range(2):
                    pb = 64 * si
                    psX = psB.tile([64, 2, 224], fp32, tag="c2", name="psX")
                    for dx in range(3):
                        rhs = bass.AP(tensor=T2.tensor,
                                      offset=pb * pitch2 + 2 * jp * 256 + dx,
                                      ap=[[pitch2, 64], [16, 2], [256, 2], [1, 224]])
                        nc.tensor.matmul(psX[:, :, :],
                                         W2[pb:pb + 64, dx, :, :], rhs,
                                         start=(dx == 0), stop=False,
                                         perf_mode=DR, tile_position=(pb, 0))
                        rhs2 = bass.AP(tensor=T2.tensor,
                                       offset=pb * pitch2 + 2 * jp * 256 + 32 + dx,
                                       ap=[[pitch2, 64], [16, 2], [256, 2], [1, 224]])
                        nc.tensor.matmul(psX[:, :, :],
                                         W2b[pb:pb + 64, dx, :, :], rhs2,
                                         start=False, stop=(dx == 2),
                                         perf_mode=DR, tile_position=(pb, 0))
                    if jp % 2 != 0:
                        p2v = bass.AP(tensor=psX.tensor, offset=0,
                                      ap=[[448, 64], [32, 14], [2, 7], [16, 2], [1, 2]])
                        S2 = tmp.tile([64, 14, 7], f16, tag="s2", name="S2")
                        nc.vector.tensor_reduce(out=S2, in_=p2v,
                                                axis=mybir.AxisListType.XY,
                                                op=ALU.max)
                        S3 = tmp.tile([64, 14, 7], fp32, tag="s3", name="S3")
                        nc.vector.tensor_scalar(out=S3, in0=S2,
                                                scalar1=CV[0:64, 0:1],
                                                scalar2=CV[0:64, 1:2],
                                                op0=ALU.mult, op1=ALU.add)
                        S4 = tmp.tile([64, 14, 7], fp32, tag="s4", name="S4")
                        nc.vector.tensor_scalar(out=S4, in0=S3, scalar1=3.0,
                                                scalar2=float(C_RND),
                                                op0=ALU.min, op1=ALU.add)
                        nc.scalar.activation(
                            out=T3[pb:pb + 64, 2 * jp:2 * jp + 2, 0:7, 0:7],
                            in_=S4, func=AF.Relu, bias=CV[0:64, 4:5], scale=1.0)
                    else:
                        # path D: ACT drains PSUM with fused BN affine + relu
                        psr = bass.AP(tensor=psX.tensor, offset=0,
                                      ap=[[448, 64], [224, 2], [16, 14], [1, 14]])
                        E1 = tmp.tile([64, 2, 14, 14], f16, tag="e1", name="E1")
                        nc.scalar.activation(out=E1, in_=psr, func=AF.Relu,
                                             bias=CV[0:64, 1:2],
                                             scale=CV[0:64, 0:1])
                        e1v = E1.rearrange("p b (y2 dy) x -> p b y2 dy x", dy=2)
                        E2 = tmp.tile([64, 2, 7, 14], f16, tag="e2", name="E2")
                        nc.vector.tensor_max(E2, e1v[:, :, :, 0, :],
                                             e1v[:, :, :, 1, :])
                        e2v = E2.rearrange("p b y (x2 dx) -> p b y x2 dx", dx=2)
                        E3 = tmp.tile([64, 2, 7, 7], f16, tag="e3", name="E3")
                        nc.vector.tensor_max(E3, e2v[:, :, :, :, 0],
                                             e2v[:, :, :, :, 1])
                        S4 = tmp.tile([64, 2, 7, 7], fp32, tag="s4b", name="S4")
                        nc.vector.tensor_scalar(out=S4, in0=E3, scalar1=3.0,
                                                scalar2=float(C_RND),
                                                op0=ALU.min, op1=ALU.add)
                        nc.scalar.activation(
                            out=T3[pb:pb + 64, 2 * jp:2 * jp + 2, 0:7, 0:7],
                            in_=S4, func=AF.Relu, bias=CV[0:64, 4:5], scale=1.0)

            # ---- conv3 (fp8 DoubleRow, VALID) + bn2 + quant ----
            pitch3 = 33 * 128
            Q3v = Q3.rearrange("p (y x) b -> p b y x", y=5)
            for q2 in range(4):
                for si in range(2):
                    pb = 64 * si
                    ps3 = psC.tile([64, 8, 8, 5], fp32, tag="mc", name="ps3")
                    for dx in range(3):
                        rhs = bass.AP(tensor=T3.tensor,
                                      offset=pb * pitch3 + 8 * q2 * 128 + dx,
                                      ap=[[pitch3, 64], [16, 2], [16, 64], [1, 5]])
                        nc.tensor.matmul(ps3[:, :, :, :],
                                         W3[pb:pb + 64, dx, :, :], rhs,
                                         start=(dx == 0), stop=False,
                                         perf_mode=DR, tile_position=(pb, 0))
                        rhs2 = bass.AP(tensor=T3.tensor,
                                       offset=pb * pitch3 + 8 * q2 * 128 + 32 + dx,
                                       ap=[[pitch3, 64], [16, 2], [16, 64], [1, 5]])
                        nc.tensor.matmul(ps3[:, :, :, :],
                                         W3b[pb:pb + 64, dx, :, :], rhs2,
                                         start=False, stop=(dx == 2),
                                         perf_mode=DR, tile_position=(pb, 0))
                    U1 = tmp.tile([64, 8, 8, 5], fp32, tag="u1", name="U1")
                    nc.scalar.activation(out=U1, in_=ps3, func=AF.Identity,
                                         bias=CV[0:64, 3:4], scale=CV[0:64, 2:3])
                    U2 = tmp.tile([64, 8, 8, 5], fp32, tag="u2", name="U2")
                    nc.vector.tensor_scalar(out=U2, in0=U1, scalar1=3.0,
                                            scalar2=float(C_RND),
                                            op0=ALU.min, op1=ALU.add)
                    nc.scalar.activation(
                        out=Q3v[pb:pb + 64, 8 * q2:8 * q2 + 8, :, :],
                        in_=U2[:, :, 0:5, :], func=AF.Relu,
                        bias=CV[0:64, 4:5], scale=1.0)

            # ---- fc1 input staging: deferred 4 DMAs (jh x set), emitted
            # after the NEXT chunk's im2col so they can't head-of-line block
            # the SP queue's prefetch ----
            def _stage(Q3=Q3, img0=img0):
                for jh in range(2):
                    nk = 13 if jh == 0 else 12
                    for si in range(2):
                        src = bass.AP(tensor=Q3.tensor,
                                      offset=64 * si * 800 + 32 * jh,
                                      ap=[[800, 64], [64, nk], [1, 32]])
                        dst = bass.AP(tensor=F2.tensor,
                                      offset=64 * jh * (14 * 512) + img0 + 32 * si,
                                      ap=[[14 * 512, 64], [512, nk], [1, 32]])
                        nc.sync.dma_start(out=dst, in_=src)
            pending_fc.append(_stage)

        while pending_fc:
            pending_fc.pop(0)()

        # ---- fc1 (fp8 DoubleRow over 7 chunk-pairs) ----
        for m in range(4):
            psf = psB.tile([128, 512], fp32, tag="c2", name="psf")
            for u in range(7):
                rhs = bass.AP(tensor=F2.tensor, offset=2 * u * 512,
                              ap=[[14 * 512, 128], [512, 2], [1, 512]])
                nc.tensor.matmul(psf, FW1[:, m, u, :, :], rhs,
                                 start=(u == 0), stop=(u == 6), perf_mode=DR)
            h = sg.tile([128, 512], fp32, name=f"H1v_{m}")
            H1[m] = h
            nc.scalar.activation(out=h, in_=psf, func=AF.Identity,
                                 bias=FB1[:, m:m + 1], scale=c13)

        # ---- fc2 (fc1 acts stationary -> logits batch-on-partitions) ----
        for qq in range(4):
            psj = psC.tile([128, 10], fp32, tag="mc", name="psj")
            for k2 in range(4):
                nc.tensor.matmul(psj, H1[k2][:, 128 * qq:128 * (qq + 1)],
                                 FW2[:, k2, :], start=(k2 == 0), stop=(k2 == 3))
            v = tmp.tile([128, 10], fp32, tag="lg", name="v")
            nc.vector.tensor_add(v, psj, FB2T)
            mx = tmp.tile([128, 1], fp32, tag="mx", name="mx")
            nc.vector.reduce_max(out=mx, in_=v, axis=AX)
            tt = tmp.tile([128, 10], fp32, tag="tt", name="tt")
            nc.vector.tensor_scalar(out=tt, in0=v, scalar1=mx, scalar2=None,
                                    op0=ALU.subtract)
            ee = tmp.tile([128, 10], fp32, tag="ee", name="ee")
            ss = tmp.tile([128, 1], fp32, tag="ss", name="ss")
            nc.scalar.activation(out=ee, in_=tt, func=AF.Exp, accum_out=ss)
            ll = tmp.tile([128, 1], fp32, tag="ll", name="ll")
            nc.scalar.activation(out=ll, in_=ss, func=AF.Ln)
            oo = tmp.tile([128, 10], fp32, tag="oo", name="oo")
            nc.vector.tensor_scalar(out=oo, in0=tt, scalar1=ll, scalar2=None,
                                    op0=ALU.subtract)
            nc.sync.dma_start(out=d_out[128 * qq:128 * (qq + 1), :], in_=oo)

    nc.finalize()
    return nc


_NC_CACHE = None
TRACE = False
TRACE_DIR = None


def kernel(**inputs):
    global _NC_CACHE
    x = np.asarray(inputs["x"], dtype=np.float32).reshape(4096, 28, 28)
    consts = _prep(**{k: v for k, v in inputs.items() if k != "x"})

    if _NC_CACHE is None:
        _NC_CACHE = _build_nc()
    nc = _NC_CACHE

    in_maps = []
    for corei in range(N_CORES):
        hi, mid, lo = _split_x(x[corei * B_CORE:(corei + 1) * B_CORE])
        m = dict(s_hi=hi, s_mid=mid, s_lo=lo)
        m.update(consts)
        in_maps.append(m)

    from concourse.bass_utils import run_bass_kernel_spmd
    res = run_bass_kernel_spmd(nc, in_maps, core_ids=list(range(N_CORES)),
                               trace=TRACE, tmpdir=TRACE_DIR)
    if res.exec_time_ns is not None:
        print(f"HW exec time: {res.exec_time_ns} ns")
        print(f"mean exec time: {res.mean_exec_time_ns} ns")
    out = np.concatenate([r["out"] for r in res.results], axis=0)
    return out.astype(np.float32)


# revision 9
# speedup vs baseline: 1.0166x; 1.0166x over previous
"""Trainium2 Bass kernel for nn_Net_12481174962824 (binarized CNN) — v2.

Data-parallel over 8 cores (512 images each). Within a core, images are
processed in chunks of 64, split into two 32-image sets living on partition
halves (set A -> partitions 0-63, set B -> 64-127) so every post-conv
pool/quant chain runs at full 128-partition width with no cross-partition
copies.

  conv1: bf16 two-split (hi/mid) im2col matmul, K=20 per 32-row group
    (2 splits x 9 taps + 2 ones-rows carrying the bias 3*b1 as hi/lo bf16
    weights; the x3 activation scale is folded into the +/-3 weights).
    4 row-groups x 2 col-groups = 8-way PE tiling, PSUM [128, 14, 28].
  pool+quant: banks alternate between a DVE pooled-reduce path and an
    ACT-drain + DVE tensor_max path to balance the two PSUM-reader engines;
    round via min(3,x)+2^23 (RNE) then ACT Relu(x-C) -> fp8.
  conv2/conv3: fp8e4 DoubleRow matmuls — activations {0..3} and +/-1
    weights are exact in fp8; each MM's in-cell pair covers taps (dy,dy+1)
    via the 16-byte row pitch, so 3 taps/column need 2 MMs per dx (6 total
    per 9-tap conv). Sets run as separate row/col tile groups.
  fc1: fp8 DoubleRow over 14 feature-chunks (zero-padded), fc2 fp32 with
    fc1 activations stationary so logits land batch-on-partitions.
"""

import numpy as np
import ml_dtypes

BF16 = ml_dtypes.bfloat16
FP8 = ml_dtypes.float8_e4m3
F32 = np.float32
C_RND = np.float32(12582912.0)  # 1.5 * 2**23: (x + C) - C rounds to int (RNE)
N_CORES = 8
B_CORE = 512
NB = 64               # images per chunk
NCHUNK = B_CORE // NB
NBG = NB // 4         # images per conv1 row-group (16)


def _f32(x):
    return np.asarray(x, dtype=np.float32)


def _prep(w1, b1, w2, g1, be1, m1, v1, w3, g2, be2, m2, v2, fw1, fb1, fw2, fb2):
    """Host prep of small weight tensors. Returns dict of np arrays."""
    sg = lambda w: np.where(_f32(w) >= 0, np.float32(1), np.float32(-1))

    # conv1 lhsT [128, 64]: rows 32g + 9s + t = 3*sign(w1); rows 32g+18/19 = bias 3*b1 hi/lo
    w1b = sg(w1)  # [64,1,3,3]
    base = (3.0 * w1b[:, 0].reshape(64, 9).T).astype(F32)  # [9 taps, 64]
    b3 = (3.0 * _f32(b1)).astype(F32)
    b3hi = b3.astype(BF16)
    b3lo = (b3 - b3hi.astype(F32)).astype(BF16)
    w1l = np.zeros((128, 64), dtype=BF16)
    for g in range(4):
        for s in range(3):
            w1l[32 * g + 9 * s: 32 * g + 9 * s + 9, :] = base.astype(BF16)
        w1l[32 * g + 27, :] = b3hi
        w1l[32 * g + 28, :] = b3lo

    # conv2/conv3 DoubleRow weights: [128, 3(dx), 2(j), 64] with rows 64-127 duplicated
    def dr_pair(w):
        wb = sg(w)  # [cout, cin, dy, dx]
        a = np.zeros((64, 3, 2, 64), dtype=FP8)
        b = np.zeros((64, 3, 2, 64), dtype=FP8)
        for dx in range(3):
            a[:, dx, 0, :] = wb[:, :, 0, dx].T.astype(FP8)
            a[:, dx, 1, :] = wb[:, :, 1, dx].T.astype(FP8)
            b[:, dx, 0, :] = wb[:, :, 2, dx].T.astype(FP8)
        a2 = np.concatenate([a, a], axis=0).reshape(128, 3 * 2 * 64)
        b2 = np.concatenate([b, b], axis=0).reshape(128, 3 * 2 * 64)
        return a2, b2

    w2l, w2l2 = dr_pair(w2)
    w3l, w3l2 = dr_pair(w3)

    # BN folds (fp32 arithmetic like the reference); activations carry a 3x scale
    def fold(g, be, m, v):
        rs = (np.float32(1.0) / np.sqrt(_f32(v) + np.float32(1e-4))).astype(F32)
        inv = (_f32(g) * rs).astype(F32)
        assert (inv > 0).all(), "negative BN scale: pool/quant commute breaks"
        s = inv.astype(F32)                                   # 3*c*inv, c=1/3
        b = (np.float32(3.0) * (_f32(be) - _f32(m) * inv)).astype(F32)
        return s, b

    s1v, b1v = fold(g1, be1, m1, v1)
    s2v, b2v = fold(g2, be2, m2, v2)
    cvec = np.zeros((128, 6), dtype=F32)
    for half in range(2):
        cvec[64 * half:64 * half + 64, 0] = s1v
        cvec[64 * half:64 * half + 64, 1] = b1v
        cvec[64 * half:64 * half + 64, 2] = s2v
        cvec[64 * half:64 * half + 64, 3] = b2v
    cvec[:, 4] = -C_RND

    # fc1 DoubleRow lhsT [128, 4(m), 7(u), 2(j), 128]:
    # partition p = 64*jh + ch holds feature (4u+2j+jh)*64 + ch
    fw1b = sg(fw1)  # [512, 1600]
    fw1l = np.zeros((128, 4, 7, 2, 128), dtype=FP8)
    for m in range(4):
        for u in range(7):
            for j in range(2):
                for jh in range(2):
                    p = 4 * u + 2 * j + jh
                    if p >= 25:
                        continue
                    fw1l[64 * jh:64 * jh + 64, m, u, j, :] = (
                        fw1b[128 * m:128 * (m + 1), 64 * p:64 * (p + 1)].T.astype(FP8))
    fw1l = fw1l.reshape(128, 4 * 7 * 2 * 128)

    # fc2 lhsT [128, 4, 10] fp32: row j of chunk k2 = fc1-feature 128*k2+j
    fw2l = np.zeros((128, 4, 10), dtype=F32)
    for k2 in range(4):
        fw2l[:, k2, :] = _f32(fw2)[:, 128 * k2:128 * (k2 + 1)].T
    fw2l = fw2l.reshape(128, 40)

    fb1v = _f32(fb1).reshape(4, 128).T.copy()  # [128, 4]
    fb2v = _f32(fb2).reshape(10, 1).copy()     # [10, 1]
    onesv = np.ones(2 * 16 * 840, dtype=BF16)
    return dict(w1l=w1l, w2l=w2l, w2l2=w2l2, w3l=w3l, w3l2=w3l2, cvec=cvec,
                fw1l=fw1l, fw2l=fw2l, fb1v=fb1v, fb2v=fb2v, onesv=onesv)


def _split_x(x_shard):
    """[512,28,28] f32 -> three padded bf16 split tensors [512*900+64]."""
    S = np.zeros((B_CORE, 30, 30), dtype=F32)
    S[:, 1:29, 1:29] = x_shard
    S = S.reshape(-1)
    hi = S.astype(BF16)
    r = (S - hi.astype(F32)).astype(F32)
    mid = r.astype(BF16)
    lo = (r - mid.astype(F32)).astype(BF16)
    pad = np.zeros(64, dtype=BF16)
    return (np.concatenate([hi, pad]), np.concatenate([mid, pad]),
            np.concatenate([lo, pad]))


def _build_nc():
    import concourse.bass as bass
    import concourse.bacc as bacc
    import concourse.tile as tile
    import concourse.mybir as mybir
    from contextlib import ExitStack

    fp32 = mybir.dt.float32
    bf16 = mybir.dt.bfloat16
    f16 = mybir.dt.float16
    f8 = mybir.dt.float8e4
    AX = mybir.AxisListType.X
    AF = mybir.ActivationFunctionType
    ALU = mybir.AluOpType
    DR = mybir.MatmulPerfMode.DoubleRow
    c13 = float(np.float32(1.0) / np.float32(3.0))

    nc = bacc.Bacc("TRN2", target_bir_lowering=False)
    d_shi = nc.dram_tensor("s_hi", [B_CORE * 900 + 64], bf16, kind="ExternalInput")
    d_smid = nc.dram_tensor("s_mid", [B_CORE * 900 + 64], bf16, kind="ExternalInput")
    d_slo = nc.dram_tensor("s_lo", [B_CORE * 900 + 64], bf16, kind="ExternalInput")
    d_w1 = nc.dram_tensor("w1l", [128, 64], bf16, kind="ExternalInput")
    d_w2 = nc.dram_tensor("w2l", [128, 384], f8, kind="ExternalInput")
    d_w2b = nc.dram_tensor("w2l2", [128, 384], f8, kind="ExternalInput")
    d_w3 = nc.dram_tensor("w3l", [128, 384], f8, kind="ExternalInput")
    d_w3b = nc.dram_tensor("w3l2", [128, 384], f8, kind="ExternalInput")
    d_cv = nc.dram_tensor("cvec", [128, 6], fp32, kind="ExternalInput")
    d_fw1 = nc.dram_tensor("fw1l", [128, 4 * 7 * 2 * 128], f8, kind="ExternalInput")
    d_fw2 = nc.dram_tensor("fw2l", [128, 40], fp32, kind="ExternalInput")
    d_fb1 = nc.dram_tensor("fb1v", [128, 4], fp32, kind="ExternalInput")
    d_fb2 = nc.dram_tensor("fb2v", [10, 1], fp32, kind="ExternalInput")
    d_ones = nc.dram_tensor("onesv", [2 * 16 * 840], bf16, kind="ExternalInput")
    d_out = nc.dram_tensor("out", [B_CORE, 10], fp32, kind="ExternalOutput")

    splits = [d_shi, d_smid, d_slo]

    with tile.TileContext(nc) as tc, ExitStack() as ctx:
        sg = ctx.enter_context(tc.tile_pool(name="sg", bufs=1))
        tmp = ctx.enter_context(tc.tile_pool(name="tmp", bufs=6))
        psA = ctx.enter_context(tc.tile_pool(name="psA", bufs=3, space="PSUM"))
        psB = ctx.enter_context(tc.tile_pool(name="psB", bufs=3, space="PSUM"))
        psC = ctx.enter_context(tc.tile_pool(name="psC", bufs=2, space="PSUM"))

        # --- weights / constants ---
        W1 = sg.tile([128, 64], bf16)
        nc.sync.dma_start(out=W1, in_=d_w1[:, :])
        W2 = sg.tile([128, 3, 2, 64], f8)
        nc.sync.dma_start(out=W2, in_=d_w2[:, :].rearrange("p (d j c) -> p d j c", d=3, j=2))
        W2b = sg.tile([128, 3, 2, 64], f8)
        nc.sync.dma_start(out=W2b, in_=d_w2b[:, :].rearrange("p (d j c) -> p d j c", d=3, j=2))
        W3 = sg.tile([128, 3, 2, 64], f8)
        nc.sync.dma_start(out=W3, in_=d_w3[:, :].rearrange("p (d j c) -> p d j c", d=3, j=2))
        W3b = sg.tile([128, 3, 2, 64], f8)
        nc.sync.dma_start(out=W3b, in_=d_w3b[:, :].rearrange("p (d j c) -> p d j c", d=3, j=2))
        CV = sg.tile([128, 6], fp32)
        nc.sync.dma_start(out=CV, in_=d_cv[:, :])
        FW1 = sg.tile([128, 4, 7, 2, 128], f8)
        nc.sync.dma_start(out=FW1, in_=d_fw1[:, :].rearrange("p (m u j c) -> p m u j c", m=4, u=7, j=2))
        FW2 = sg.tile([128, 4, 10], fp32)
        nc.sync.dma_start(out=FW2, in_=d_fw2[:, :].rearrange("p (k m) -> p k m", k=4))
        FB1 = sg.tile([128, 4], fp32)
        nc.sync.dma_start(out=FB1, in_=d_fb1[:, :])
        FB2T = sg.tile([128, 10], fp32)
        fb2b = bass.AP(tensor=d_fb2[:, :].tensor, offset=0, ap=[[0, 128], [1, 10]])
        nc.sync.dma_start(out=FB2T, in_=fb2b)

        # --- persistent ping-pong activation tiles ---
        T1s = [sg.tile([128, 16, 840], bf16, name=f"T1_{i}") for i in range(2)]
        T2s = [sg.tile([128, 33, 16, 16], f8, name=f"T2_{i}") for i in range(2)]
        T3s = [sg.tile([128, 33, 8, 16], f8, name=f"T3_{i}") for i in range(2)]
        Q3s = [sg.tile([128, 25, 32], f8, name=f"Q3_{i}") for i in range(2)]
        F2 = sg.tile([128, 14, 512], f8)
        H1 = [None] * 4

        # one-time pads/ones
        for i in range(2):
            T2, T3 = T2s[i], T3s[i]
            nc.gpsimd.memset(T2[:, :, 0, :], 0)
            nc.gpsimd.memset(T2[:, :, 15, :], 0)
            nc.gpsimd.memset(T2[:, :, 1:15, 0], 0)
            nc.gpsimd.memset(T2[:, :, 1:15, 15], 0)
            nc.gpsimd.memset(T2[:, 32, 1:15, 1:15], 0)
            nc.gpsimd.memset(T3[:, :, 7, :], 0)
            nc.gpsimd.memset(T3[:, :, 0:7, 7:16], 0)
            nc.gpsimd.memset(T3[:, 32, :, :], 0)
            for g in range(4):
                nc.sync.dma_start(
                    out=T1s[i][32 * g + 27:32 * g + 29, :, :].rearrange("p b x -> p (b x)"),
                    in_=bass.AP(tensor=d_ones[:].tensor, offset=0,
                                ap=[[13440, 2], [1, 13440]]))
        nc.vector.memset(F2[64:128, 12, :], 0)
        nc.vector.memset(F2[:, 13, :], 0)
        # touch Exp/Ln act tables now so the loads don't stall the softmax tail
        warm = sg.tile([128, 2], fp32, name="warm")
        nc.gpsimd.memset(warm, 1.0)
        nc.scalar.activation(out=warm[:, 0:1], in_=warm[:, 0:1], func=AF.Exp)
        nc.scalar.activation(out=warm[:, 1:2], in_=warm[:, 1:2], func=AF.Ln)

        pending_fc = []
        for ch in range(NCHUNK):
            img0 = ch * NB
            T1, T2, T3, Q3 = T1s[ch % 2], T2s[ch % 2], T3s[ch % 2], Q3s[ch % 2]

            # ---- conv1 im2col DMAs: one per (group, split, dy) ----
            for g in range(4):
                for s in range(3):
                    for dy in range(3):
                        src = bass.AP(
                            tensor=splits[s][:].tensor,
                            offset=(img0 + 16 * g) * 900 + 30 * dy,
                            ap=[[1, 3], [900, 16], [1, 840]])
                        r0 = 32 * g + 9 * s + 3 * dy
                        # lo split goes via the Pool engine's SWDGE queue so
                        # the SP/HWDGE path only carries 24 DMAs per chunk
                        if s == 2:
                            nc.gpsimd.dma_start(out=T1[r0:r0 + 3, :, :], in_=src)
                        else:
                            nc.sync.dma_start(out=T1[r0:r0 + 3, :, :], in_=src)
            while pending_fc:
                pending_fc.pop(0)()

            T1v = T1.rearrange("p b (h y x) -> p b h y x", h=2, y=14, x=30)

            # ---- conv1 + pool + quant ----
            for bb in range(16):
                for half in range(2):
                    ys = 1 + 7 * half
                    P0 = psA.tile([128, 14, 28], fp32, tag="c1", name="P0")
                    P1 = psA.tile([128, 14, 28], fp32, tag="c1", name="P1")
                    for g in range(4):
                        P = (P0, P1)[g % 2]
                        colg = 64 * (g // 2)
                        nc.tensor.matmul(
                            P[colg:colg + 64, :, :],
                            W1[32 * g:32 * g + 29, :],
                            T1v[32 * g:32 * g + 29, bb, half, :, 0:28],
                            start=True, stop=True, skip_group_check=True,
                            tile_position=(32 * g, colg))
                    R2P = tmp.tile([128, 2, 7, 14], fp32, tag="r2p", name="R2P")
                    if bb % 2 != 0:
                        # path R: DVE pooled-reduce straight from PSUM
                        for pi, P in enumerate((P0, P1)):
                            psv = P.rearrange(
                                "p (y2 dy) (x2 dx) -> p y2 x2 dy dx",
                                dy=2, dx=2)
                            nc.vector.tensor_reduce(out=R2P[:, pi, :, :],
                                                    in_=psv,
                                                    axis=mybir.AxisListType.XY,
                                                    op=ALU.max)
                    else:
                        # path D: ACT drains PSUM so DVE only does the
                        # cheap SBUF max pair + round (drain-load balance)
                        for pi, P in enumerate((P0, P1)):
                            D1 = tmp.tile([128, 14, 28], fp32, tag="d1", name="D1")
                            nc.scalar.activation(out=D1, in_=P, func=AF.Copy,
                                                 bias=0.0, scale=1.0)
                            d1v = D1.rearrange("p (y2 dy) x -> p y2 dy x", dy=2)
                            D2 = tmp.tile([128, 7, 28], fp32, tag="d2", name="D2")
                            nc.vector.tensor_max(D2, d1v[:, :, 0, :],
                                                 d1v[:, :, 1, :])
                            d2v = D2.rearrange("p y (x2 dx) -> p y x2 dx", dx=2)
                            nc.vector.tensor_max(R2P[:, pi, :, :],
                                                 d2v[:, :, :, 0],
                                                 d2v[:, :, :, 1])
                    # shared round + single paired write into slots bb, bb+16
                    R3P = tmp.tile([128, 2, 7, 14], fp32, tag="r3p", name="R3P")
                    nc.vector.tensor_scalar(out=R3P, in0=R2P, scalar1=3.0,
                                            scalar2=float(C_RND),
                                            op0=ALU.min, op1=ALU.add)
                    T2p = T2[:, 0:32, :, :].rearrange(
                        "p (b2 sl) y x -> p sl b2 y x", b2=2)
                    nc.scalar.activation(out=T2p[:, bb, :, ys:ys + 7, 1:15],
                                         in_=R3P, func=AF.Relu,
                                         bias=CV[:, 4:5], scale=1.0)

            # ---- conv2 (fp8 DoubleRow, 6 MMs per set, per-set dst banks) ----
            pitch2 = 33 * 256
            for jp in range(16):
                for si in range(2):
                    pb = 64 * si
                    psX = psB.tile([64, 2, 224], fp32, tag="c2", name="psX")
                    for dx in range(3):
                        rhs = bass.AP(tensor=T2.tensor,
                                      offset=pb * pitch2 + 2 * jp * 256 + dx,
                                      ap=[[pitch2, 64], [16, 2], [256, 2], [1, 224]])
                        nc.tensor.matmul(psX[:, :, :],
                                         W2[pb:pb + 64, dx, :, :], rhs,
                                         start=(dx == 0), stop=False,
                                         perf_mode=DR, tile_position=(pb, 0))
                        rhs2 = bass.AP(tensor=T2.tensor,
                                       offset=pb * pitch2 + 2 * jp * 256 + 32 + dx,
                                       ap=[[pitch2, 64], [16, 2], [256, 2], [1, 224]])
                        nc.tensor.matmul(psX[:, :, :],
                                         W2b[pb:pb + 64, dx, :, :], rhs2,
                                         start=False, stop=(dx == 2),
                                         perf_mode=DR, tile_position=(pb, 0))
                    if jp % 2 != 0:
                        p2v = bass.AP(tensor=psX.tensor, offset=0,
                                      ap=[[448, 64], [32, 14], [2, 7], [16, 2], [1, 2]])
                        S2 = tmp.tile([64, 14, 7], f16, tag="s2", name="S2")
                        nc.vector.tensor_reduce(out=S2, in_=p2v,
                                                axis=mybir.AxisListType.XY,
                                                op=ALU.max)
                        S3 = tmp.tile([64, 14, 7], fp32, tag="s3", name="S3")
                        nc.vector.tensor_scalar(out=S3, in0=S2,
                                                scalar1=CV[0:64, 0:1],
                                                scalar2=CV[0:64, 1:2],
                                                op0=ALU.mult, op1=ALU.add)
                        S4 = tmp.tile([64, 14, 7], fp32, tag="s4", name="S4")
                        nc.vector.tensor_scalar(out=S4, in0=S3, scalar1=3.0,
                                                scalar2=float(C_RND),
                                                op0=ALU.min, op1=ALU.add)
                        nc.scalar.activation(
                            out=T3[pb:pb + 64, 2 * jp:2 * jp + 2, 0:7, 0:7],
                            in_=S4, func=AF.Relu, bias=CV[0:64, 4:5], scale=1.0)
                    else:
                        # path D: ACT drains PSUM with fused BN affine + relu
                        psr = bass.AP(tensor=psX.tensor, offset=0,
                                      ap=[[448, 64], [224, 2], [16, 14], [1, 14]])
                        E1 = tmp.tile([64, 2, 14, 14], f16, tag="e1", name="E1")
                        nc.scalar.activation(out=E1, in_=psr, func=AF.Relu,
                                             bias=CV[0:64, 1:2],
                                             scale=CV[0:64, 0:1])
                        e1v = E1.rearrange("p b (y2 dy) x -> p b y2 dy x", dy=2)
                        E2 = tmp.tile([64, 2, 7, 14], f16, tag="e2", name="E2")
                        nc.vector.tensor_max(E2, e1v[:, :, :, 0, :],
                                             e1v[:, :, :, 1, :])
                        e2v = E2.rearrange("p b y (x2 dx) -> p b y x2 dx", dx=2)
                        E3 = tmp.tile([64, 2, 7, 7], f16, tag="e3", name="E3")
                        nc.vector.tensor_max(E3, e2v[:, :, :, :, 0],
                                             e2v[:, :, :, :, 1])
                        S4 = tmp.tile([64, 2, 7, 7], fp32, tag="s4b", name="S4")
                        nc.vector.tensor_scalar(out=S4, in0=E3, scalar1=3.0,
                                                scalar2=float(C_RND),
                                                op0=ALU.min, op1=ALU.add)
                        nc.scalar.activation(
                            out=T3[pb:pb + 64, 2 * jp:2 * jp + 2, 0:7, 0:7],
                            in_=S4, func=AF.Relu, bias=CV[0:64, 4:5], scale=1.0)

            # ---- conv3 (fp8 DoubleRow, VALID) + bn2 + quant ----
            pitch3 = 33 * 128
            Q3v = Q3.rearrange("p (y x) b -> p b y x", y=5)
            for q2 in range(4):
                for si in range(2):
                    pb = 64 * si
                    ps3 = psC.tile([64, 8, 8, 5], fp32, tag="mc", name="ps3")
                    for dx in range(3):
                        rhs = bass.AP(tensor=T3.tensor,
                                      offset=pb * pitch3 + 8 * q2 * 128 + dx,
                                      ap=[[pitch3, 64], [16, 2], [16, 64], [1, 5]])
                        nc.tensor.matmul(ps3[:, :, :, :],
                                         W3[pb:pb + 64, dx, :, :], rhs,
                                         start=(dx == 0), stop=False,
                                         perf_mode=DR, tile_position=(pb, 0))
                        rhs2 = bass.AP(tensor=T3.tensor,
                                       offset=pb * pitch3 + 8 * q2 * 128 + 32 + dx,
                                       ap=[[pitch3, 64], [16, 2], [16, 64], [1, 5]])
                        nc.tensor.matmul(ps3[:, :, :, :],
                                         W3b[pb:pb + 64, dx, :, :], rhs2,
                                         start=False, stop=(dx == 2),
                                         perf_mode=DR, tile_position=(pb, 0))
                    U1 = tmp.tile([64, 8, 8, 5], fp32, tag="u1", name="U1")
                    nc.scalar.activation(out=U1, in_=ps3, func=AF.Identity,
                                         bias=CV[0:64, 3:4], scale=CV[0:64, 2:3])
                    U2 = tmp.tile([64, 8, 8, 5], fp32, tag="u2", name="U2")
                    nc.vector.tensor_scalar(out=U2, in0=U1, scalar1=3.0,
                                            scalar2=float(C_RND),
                                            op0=ALU.min, op1=ALU.add)
                    nc.scalar.activation(
                        out=Q3v[pb:pb + 64, 8 * q2:8 * q2 + 8, :, :],
                        in_=U2[:, :, 0:5, :], func=AF.Relu,
                        bias=CV[0:64, 4:5], scale=1.0)

            # ---- fc1 input staging: deferred 4 DMAs (jh x set), emitted
            # after the NEXT chunk's im2col so they can't head-of-line block
            # the SP queue's prefetch ----
            def _stage(Q3=Q3, img0=img0):
                for jh in range(2):
                    nk = 13 if jh == 0 else 12
                    for si in range(2):
                        src = bass.AP(tensor=Q3.tensor,
                                      offset=64 * si * 800 + 32 * jh,
                                      ap=[[800, 64], [64, nk], [1, 32]])
                        dst = bass.AP(tensor=F2.tensor,
                                      offset=64 * jh * (14 * 512) + img0 + 32 * si,
                                      ap=[[14 * 512, 64], [512, nk], [1, 32]])
                        nc.sync.dma_start(out=dst, in_=src)
            pending_fc.append(_stage)

        while pending_fc:
            pending_fc.pop(0)()

        # ---- fc1 (fp8 DoubleRow over 7 chunk-pairs) ----
        for m in range(4):
            psf = psB.tile([128, 512], fp32, tag="c2", name="psf")
            for u in range(7):
                rhs = bass.AP(tensor=F2.tensor, offset=2 * u * 512,
                              ap=[[14 * 512, 128], [512, 2], [1, 512]])
                nc.tensor.matmul(psf, FW1[:, m, u, :, :], rhs,
                                 start=(u == 0), stop=(u == 6), perf_mode=DR)
            h = sg.tile([128, 512], fp32, name=f"H1v_{m}")
            H1[m] = h
            nc.scalar.activation(out=h, in_=psf, func=AF.Identity,
                                 bias=FB1[:, m:m + 1], scale=c13)

        # ---- fc2 (fc1 acts stationary -> logits batch-on-partitions) ----
        for qq in range(4):
            psj = psC.tile([128, 10], fp32, tag="mc", name="psj")
            for k2 in range(4):
                nc.tensor.matmul(psj, H1[k2][:, 128 * qq:128 * (qq + 1)],
                                 FW2[:, k2, :], start=(k2 == 0), stop=(k2 == 3))
            v = tmp.tile([128, 10], fp32, tag="lg", name="v")
            nc.vector.tensor_add(v, psj, FB2T)
            mx = tmp.tile([128, 1], fp32, tag="mx", name="mx")
            nc.vector.reduce_max(out=mx, in_=v, axis=AX)
            tt = tmp.tile([128, 10], fp32, tag="tt", name="tt")
            nc.vector.tensor_scalar(out=tt, in0=v, scalar1=mx, scalar2=None,
                                    op0=ALU.subtract)
            ee = tmp.tile([128, 10], fp32, tag="ee", name="ee")
            ss = tmp.tile([128, 1], fp32, tag="ss", name="ss")
            nc.scalar.activation(out=ee, in_=tt, func=AF.Exp, accum_out=ss)
            ll = tmp.tile([128, 1], fp32, tag="ll", name="ll")
            nc.scalar.activation(out=ll, in_=ss, func=AF.Ln)
            oo = tmp.tile([128, 10], fp32, tag="oo", name="oo")
            nc.vector.tensor_scalar(out=oo, in0=tt, scalar1=ll, scalar2=None,
                                    op0=ALU.subtract)
            nc.sync.dma_start(out=d_out[128 * qq:128 * (qq + 1), :], in_=oo)

    nc.finalize()
    return nc


_NC_CACHE = None
TRACE = False
TRACE_DIR = None


def kernel(**inputs):
    global _NC_CACHE
    x = np.asarray(inputs["x"], dtype=np.float32).reshape(4096, 28, 28)
    consts = _prep(**{k: v for k, v in inputs.items() if k != "x"})

    if _NC_CACHE is None:
        _NC_CACHE = _build_nc()
    nc = _NC_CACHE

    in_maps = []
    for corei in range(N_CORES):
        hi, mid, lo = _split_x(x[corei * B_CORE:(corei + 1) * B_CORE])
        m = dict(s_hi=hi, s_mid=mid, s_lo=lo)
        m.update(consts)
        in_maps.append(m)

    from concourse.bass_utils import run_bass_kernel_spmd
    res = run_bass_kernel_spmd(nc, in_maps, core_ids=list(range(N_CORES)),
                               trace=TRACE, tmpdir=TRACE_DIR)
    if res.exec_time_ns is not None:
        print(f"HW exec time: {res.exec_time_ns} ns")
        print(f"mean exec time: {res.mean_exec_time_ns} ns")
    out = np.concatenate([r["out"] for r in res.results], axis=0)
    return out.astype(np.float32)


# revision 10
# speedup vs baseline: 1.0169x; 1.0003x over previous
"""Trainium2 Bass kernel for nn_Net_12481174962824 (binarized CNN) — v2.

Data-parallel over 8 cores (512 images each). Within a core, images are
processed in chunks of 64, split into two 32-image sets living on partition
halves (set A -> partitions 0-63, set B -> 64-127) so every post-conv
pool/quant chain runs at full 128-partition width with no cross-partition
copies.

  conv1: bf16 two-split (hi/mid) im2col matmul, K=20 per 32-row group
    (2 splits x 9 taps + 2 ones-rows carrying the bias 3*b1 as hi/lo bf16
    weights; the x3 activation scale is folded into the +/-3 weights).
    4 row-groups x 2 col-groups = 8-way PE tiling, PSUM [128, 14, 28].
  pool+quant: banks alternate between a DVE pooled-reduce path and an
    ACT-drain + DVE tensor_max path to balance the two PSUM-reader engines;
    round via min(3,x)+2^23 (RNE) then ACT Relu(x-C) -> fp8.
  conv2/conv3: fp8e4 DoubleRow matmuls — activations {0..3} and +/-1
    weights are exact in fp8; each MM's in-cell pair covers taps (dy,dy+1)
    via the 16-byte row pitch, so 3 taps/column need 2 MMs per dx (6 total
    per 9-tap conv). Sets run as separate row/col tile groups.
  fc1: fp8 DoubleRow over 14 feature-chunks (zero-padded), fc2 fp32 with
    fc1 activations stationary so logits land batch-on-partitions.
"""

import numpy as np
import ml_dtypes

BF16 = ml_dtypes.bfloat16
FP8 = ml_dtypes.float8_e4m3
F32 = np.float32
C_RND = np.float32(12582912.0)  # 1.5 * 2**23: (x + C) - C rounds to int (RNE)
N_CORES = 8
B_CORE = 512
NB = 64               # images per chunk
NCHUNK = B_CORE // NB
NBG = NB // 4         # images per conv1 row-group (16)


def _f32(x):
    return np.asarray(x, dtype=np.float32)


def _prep(w1, b1, w2, g1, be1, m1, v1, w3, g2, be2, m2, v2, fw1, fb1, fw2, fb2):
    """Host prep of small weight tensors. Returns dict of np arrays."""
    sg = lambda w: np.where(_f32(w) >= 0, np.float32(1), np.float32(-1))

    # conv1 lhsT [128, 64]: rows 32g + 9s + t = 3*sign(w1); rows 32g+18/19 = bias 3*b1 hi/lo
    w1b = sg(w1)  # [64,1,3,3]
    base = (3.0 * w1b[:, 0].reshape(64, 9).T).astype(F32)  # [9 taps, 64]
    b3 = (3.0 * _f32(b1)).astype(F32)
    b3hi = b3.astype(BF16)
    b3lo = (b3 - b3hi.astype(F32)).astype(BF16)
    w1l = np.zeros((128, 64), dtype=BF16)
    for g in range(4):
        for s in range(3):
            w1l[32 * g + 9 * s: 32 * g + 9 * s + 9, :] = base.astype(BF16)
        w1l[32 * g + 27, :] = b3hi
        w1l[32 * g + 28, :] = b3lo

    # conv2/conv3 DoubleRow weights: [128, 3(dx), 2(j), 64] with rows 64-127 duplicated
    def dr_pair(w):
        wb = sg(w)  # [cout, cin, dy, dx]
        a = np.zeros((64, 3, 2, 64), dtype=FP8)
        b = np.zeros((64, 3, 2, 64), dtype=FP8)
        for dx in range(3):
            a[:, dx, 0, :] = wb[:, :, 0, dx].T.astype(FP8)
            a[:, dx, 1, :] = wb[:, :, 1, dx].T.astype(FP8)
            b[:, dx, 0, :] = wb[:, :, 2, dx].T.astype(FP8)
        a2 = np.concatenate([a, a], axis=0).reshape(128, 3 * 2 * 64)
        b2 = np.concatenate([b, b], axis=0).reshape(128, 3 * 2 * 64)
        return a2, b2

    w2l, w2l2 = dr_pair(w2)
    w3l, w3l2 = dr_pair(w3)

    # BN folds (fp32 arithmetic like the reference); activations carry a 3x scale
    def fold(g, be, m, v):
        rs = (np.float32(1.0) / np.sqrt(_f32(v) + np.float32(1e-4))).astype(F32)
        inv = (_f32(g) * rs).astype(F32)
        assert (inv > 0).all(), "negative BN scale: pool/quant commute breaks"
        s = inv.astype(F32)                                   # 3*c*inv, c=1/3
        b = (np.float32(3.0) * (_f32(be) - _f32(m) * inv)).astype(F32)
        return s, b

    s1v, b1v = fold(g1, be1, m1, v1)
    s2v, b2v = fold(g2, be2, m2, v2)
    cvec = np.zeros((128, 6), dtype=F32)
    for half in range(2):
        cvec[64 * half:64 * half + 64, 0] = s1v
        cvec[64 * half:64 * half + 64, 1] = b1v
        cvec[64 * half:64 * half + 64, 2] = s2v
        cvec[64 * half:64 * half + 64, 3] = b2v
    cvec[:, 4] = -C_RND

    # fc1 DoubleRow lhsT [128, 4(m), 7(u), 2(j), 128]:
    # partition p = 64*jh + ch holds feature (4u+2j+jh)*64 + ch
    fw1b = sg(fw1)  # [512, 1600]
    fw1l = np.zeros((128, 4, 7, 2, 128), dtype=FP8)
    for m in range(4):
        for u in range(7):
            for j in range(2):
                for jh in range(2):
                    p = 4 * u + 2 * j + jh
                    if p >= 25:
                        continue
                    fw1l[64 * jh:64 * jh + 64, m, u, j, :] = (
                        fw1b[128 * m:128 * (m + 1), 64 * p:64 * (p + 1)].T.astype(FP8))
    fw1l = fw1l.reshape(128, 4 * 7 * 2 * 128)

    # fc2 lhsT [128, 4, 10] fp32: row j of chunk k2 = fc1-feature 128*k2+j
    fw2l = np.zeros((128, 4, 10), dtype=F32)
    for k2 in range(4):
        fw2l[:, k2, :] = _f32(fw2)[:, 128 * k2:128 * (k2 + 1)].T
    fw2l = fw2l.reshape(128, 40)

    fb1v = _f32(fb1).reshape(4, 128).T.copy()  # [128, 4]
    fb2v = _f32(fb2).reshape(10, 1).copy()     # [10, 1]
    onesv = np.ones(2 * 16 * 840, dtype=BF16)
    return dict(w1l=w1l, w2l=w2l, w2l2=w2l2, w3l=w3l, w3l2=w3l2, cvec=cvec,
                fw1l=fw1l, fw2l=fw2l, fb1v=fb1v, fb2v=fb2v, onesv=onesv)


def _split_x(x_shard):
    """[512,28,28] f32 -> three padded bf16 split tensors [512*900+64]."""
    S = np.zeros((B_CORE, 30, 30), dtype=F32)
    S[:, 1:29, 1:29] = x_shard
    S = S.reshape(-1)
    hi = S.astype(BF16)
    r = (S - hi.astype(F32)).astype(F32)
    mid = r.astype(BF16)
    lo = (r - mid.astype(F32)).astype(BF16)
    pad = np.zeros(64, dtype=BF16)
    return (np.concatenate([hi, pad]), np.concatenate([mid, pad]),
            np.concatenate([lo, pad]))


def _build_nc():
    import concourse.bass as bass
    import concourse.bacc as bacc
    import concourse.tile as tile
    import concourse.mybir as mybir
    from contextlib import ExitStack

    fp32 = mybir.dt.float32
    bf16 = mybir.dt.bfloat16
    f16 = mybir.dt.float16
    f8 = mybir.dt.float8e4
    AX = mybir.AxisListType.X
    AF = mybir.ActivationFunctionType
    ALU = mybir.AluOpType
    DR = mybir.MatmulPerfMode.DoubleRow
    c13 = float(np.float32(1.0) / np.float32(3.0))

    nc = bacc.Bacc("TRN2", target_bir_lowering=False)
    d_shi = nc.dram_tensor("s_hi", [B_CORE * 900 + 64], bf16, kind="ExternalInput")
    d_smid = nc.dram_tensor("s_mid", [B_CORE * 900 + 64], bf16, kind="ExternalInput")
    d_slo = nc.dram_tensor("s_lo", [B_CORE * 900 + 64], bf16, kind="ExternalInput")
    d_w1 = nc.dram_tensor("w1l", [128, 64], bf16, kind="ExternalInput")
    d_w2 = nc.dram_tensor("w2l", [128, 384], f8, kind="ExternalInput")
    d_w2b = nc.dram_tensor("w2l2", [128, 384], f8, kind="ExternalInput")
    d_w3 = nc.dram_tensor("w3l", [128, 384], f8, kind="ExternalInput")
    d_w3b = nc.dram_tensor("w3l2", [128, 384], f8, kind="ExternalInput")
    d_cv = nc.dram_tensor("cvec", [128, 6], fp32, kind="ExternalInput")
    d_fw1 = nc.dram_tensor("fw1l", [128, 4 * 7 * 2 * 128], f8, kind="ExternalInput")
    d_fw2 = nc.dram_tensor("fw2l", [128, 40], fp32, kind="ExternalInput")
    d_fb1 = nc.dram_tensor("fb1v", [128, 4], fp32, kind="ExternalInput")
    d_fb2 = nc.dram_tensor("fb2v", [10, 1], fp32, kind="ExternalInput")
    d_ones = nc.dram_tensor("onesv", [2 * 16 * 840], bf16, kind="ExternalInput")
    d_out = nc.dram_tensor("out", [B_CORE, 10], fp32, kind="ExternalOutput")

    splits = [d_shi, d_smid, d_slo]

    with tile.TileContext(nc) as tc, ExitStack() as ctx:
        sg = ctx.enter_context(tc.tile_pool(name="sg", bufs=1))
        tmp = ctx.enter_context(tc.tile_pool(name="tmp", bufs=6))
        psA = ctx.enter_context(tc.tile_pool(name="psA", bufs=3, space="PSUM"))
        psB = ctx.enter_context(tc.tile_pool(name="psB", bufs=3, space="PSUM"))
        psC = ctx.enter_context(tc.tile_pool(name="psC", bufs=2, space="PSUM"))

        # --- weights / constants ---
        W1 = sg.tile([128, 64], bf16)
        nc.sync.dma_start(out=W1, in_=d_w1[:, :])
        W2 = sg.tile([128, 3, 2, 64], f8)
        nc.sync.dma_start(out=W2, in_=d_w2[:, :].rearrange("p (d j c) -> p d j c", d=3, j=2))
        W2b = sg.tile([128, 3, 2, 64], f8)
        nc.sync.dma_start(out=W2b, in_=d_w2b[:, :].rearrange("p (d j c) -> p d j c", d=3, j=2))
        W3 = sg.tile([128, 3, 2, 64], f8)
        nc.sync.dma_start(out=W3, in_=d_w3[:, :].rearrange("p (d j c) -> p d j c", d=3, j=2))
        W3b = sg.tile([128, 3, 2, 64], f8)
        nc.sync.dma_start(out=W3b, in_=d_w3b[:, :].rearrange("p (d j c) -> p d j c", d=3, j=2))
        CV = sg.tile([128, 6], fp32)
        nc.sync.dma_start(out=CV, in_=d_cv[:, :])
        FW1 = sg.tile([128, 4, 7, 2, 128], f8)
        FW2 = sg.tile([128, 4, 10], fp32)
        FB1 = sg.tile([128, 4], fp32)
        FB2T = sg.tile([128, 10], fp32)

        # --- persistent ping-pong activation tiles ---
        T1s = [sg.tile([128, 16, 840], bf16, name=f"T1_{i}") for i in range(2)]
        T2s = [sg.tile([128, 33, 16, 16], f8, name=f"T2_{i}") for i in range(2)]
        T3s = [sg.tile([128, 33, 8, 16], f8, name=f"T3_{i}") for i in range(2)]
        Q3s = [sg.tile([128, 25, 32], f8, name=f"Q3_{i}") for i in range(2)]
        F2 = sg.tile([128, 14, 512], f8)
        H1 = [None] * 4

        # one-time pads/ones
        for i in range(2):
            T2, T3 = T2s[i], T3s[i]
            nc.gpsimd.memset(T2[:, :, 0, :], 0)
            nc.gpsimd.memset(T2[:, :, 15, :], 0)
            nc.gpsimd.memset(T2[:, :, 1:15, 0], 0)
            nc.gpsimd.memset(T2[:, :, 1:15, 15], 0)
            nc.gpsimd.memset(T2[:, 32, 1:15, 1:15], 0)
            nc.gpsimd.memset(T3[:, :, 7, :], 0)
            nc.gpsimd.memset(T3[:, :, 0:7, 7:16], 0)
            nc.gpsimd.memset(T3[:, 32, :, :], 0)
            if i == 0:
                for g in range(4):
                    nc.sync.dma_start(
                        out=T1s[i][32 * g + 27:32 * g + 29, :, :].rearrange("p b x -> p (b x)"),
                        in_=bass.AP(tensor=d_ones[:].tensor, offset=0,
                                    ap=[[13440, 2], [1, 13440]]))
        nc.vector.memset(F2[64:128, 12, :], 0)
        nc.vector.memset(F2[:, 13, :], 0)
        # touch Exp/Ln act tables now so the loads don't stall the softmax tail
        warm = sg.tile([128, 2], fp32, name="warm")
        nc.gpsimd.memset(warm, 1.0)
        nc.scalar.activation(out=warm[:, 0:1], in_=warm[:, 0:1], func=AF.Exp)
        nc.scalar.activation(out=warm[:, 1:2], in_=warm[:, 1:2], func=AF.Ln)

        pending_fc = []
        for ch in range(NCHUNK):
            img0 = ch * NB
            T1, T2, T3, Q3 = T1s[ch % 2], T2s[ch % 2], T3s[ch % 2], Q3s[ch % 2]

            # ---- conv1 im2col DMAs: one per (group, split, dy) ----
            for g in range(4):
                for s in range(3):
                    for dy in range(3):
                        src = bass.AP(
                            tensor=splits[s][:].tensor,
                            offset=(img0 + 16 * g) * 900 + 30 * dy,
                            ap=[[1, 3], [900, 16], [1, 840]])
                        r0 = 32 * g + 9 * s + 3 * dy
                        # lo split goes via the Pool engine's SWDGE queue so
                        # the SP/HWDGE path only carries 24 DMAs per chunk
                        if s == 2:
                            nc.gpsimd.dma_start(out=T1[r0:r0 + 3, :, :], in_=src)
                        else:
                            nc.sync.dma_start(out=T1[r0:r0 + 3, :, :], in_=src)
            while pending_fc:
                pending_fc.pop(0)()
            if ch == 0:
                # deferred loads: not needed until chunk 1 / fc, so they sit
                # behind chunk 0's im2col prefetch in the SP queue
                for g in range(4):
                    nc.sync.dma_start(
                        out=T1s[1][32 * g + 27:32 * g + 29, :, :].rearrange("p b x -> p (b x)"),
                        in_=bass.AP(tensor=d_ones[:].tensor, offset=0,
                                    ap=[[13440, 2], [1, 13440]]))
                nc.sync.dma_start(out=FW1, in_=d_fw1[:, :].rearrange(
                    "p (m u j c) -> p m u j c", m=4, u=7, j=2))
                nc.sync.dma_start(out=FW2, in_=d_fw2[:, :].rearrange(
                    "p (k m) -> p k m", k=4))
                nc.sync.dma_start(out=FB1, in_=d_fb1[:, :])
                fb2b = bass.AP(tensor=d_fb2[:, :].tensor, offset=0,
                               ap=[[0, 128], [1, 10]])
                nc.sync.dma_start(out=FB2T, in_=fb2b)

            T1v = T1.rearrange("p b (h y x) -> p b h y x", h=2, y=14, x=30)

            # ---- conv1 + pool + quant ----
            for bb in range(16):
                for half in range(2):
                    ys = 1 + 7 * half
                    P0 = psA.tile([128, 14, 28], fp32, tag="c1", name="P0")
                    P1 = psA.tile([128, 14, 28], fp32, tag="c1", name="P1")
                    for g in range(4):
                        P = (P0, P1)[g % 2]
                        colg = 64 * (g // 2)
                        nc.tensor.matmul(
                            P[colg:colg + 64, :, :],
                            W1[32 * g:32 * g + 29, :],
                            T1v[32 * g:32 * g + 29, bb, half, :, 0:28],
                            start=True, stop=True, skip_group_check=True,
                            tile_position=(32 * g, colg))
                    R2P = tmp.tile([128, 2, 7, 14], fp32, tag="r2p", name="R2P")
                    if bb % 2 != 0:
                        # path R: DVE pooled-reduce straight from PSUM
                        for pi, P in enumerate((P0, P1)):
                            psv = P.rearrange(
                                "p (y2 dy) (x2 dx) -> p y2 x2 dy dx",
                                dy=2, dx=2)
                            nc.vector.tensor_reduce(out=R2P[:, pi, :, :],
                                                    in_=psv,
                                                    axis=mybir.AxisListType.XY,
                                                    op=ALU.max)
                    else:
                        # path D: ACT drains PSUM so DVE only does the
                        # cheap SBUF max pair + round (drain-load balance)
                        for pi, P in enumerate((P0, P1)):
                            D1 = tmp.tile([128, 14, 28], fp32, tag="d1", name="D1")
                            nc.scalar.activation(out=D1, in_=P, func=AF.Copy,
                                                 bias=0.0, scale=1.0)
                            d1v = D1.rearrange("p (y2 dy) x -> p y2 dy x", dy=2)
                            D2 = tmp.tile([128, 7, 28], fp32, tag="d2", name="D2")
                            nc.vector.tensor_max(D2, d1v[:, :, 0, :],
                                                 d1v[:, :, 1, :])
                            d2v = D2.rearrange("p y (x2 dx) -> p y x2 dx", dx=2)
                            nc.vector.tensor_max(R2P[:, pi, :, :],
                                                 d2v[:, :, :, 0],
                                                 d2v[:, :, :, 1])
                    # shared round + single paired write into slots bb, bb+16
                    R3P = tmp.tile([128, 2, 7, 14], fp32, tag="r3p", name="R3P")
                    nc.vector.tensor_scalar(out=R3P, in0=R2P, scalar1=3.0,
                                            scalar2=float(C_RND),
                                            op0=ALU.min, op1=ALU.add)
                    T2p = T2[:, 0:32, :, :].rearrange(
                        "p (b2 sl) y x -> p sl b2 y x", b2=2)
                    nc.scalar.activation(out=T2p[:, bb, :, ys:ys + 7, 1:15],
                                         in_=R3P, func=AF.Relu,
                                         bias=CV[:, 4:5], scale=1.0)

            # ---- conv2 (fp8 DoubleRow, 6 MMs per set, per-set dst banks) ----
            pitch2 = 33 * 256
            for jp in range(16):
                for si in range(2):
                    pb = 64 * si
                    psX = psB.tile([64, 2, 224], fp32, tag="c2", name="psX")
                    for dx in range(3):
                        rhs = bass.AP(tensor=T2.tensor,
                                      offset=pb * pitch2 + 2 * jp * 256 + dx,
                                      ap=[[pitch2, 64], [16, 2], [256, 2], [1, 224]])
                        nc.tensor.matmul(psX[:, :, :],
                                         W2[pb:pb + 64, dx, :, :], rhs,
                                         start=(dx == 0), stop=False,
                                         perf_mode=DR, tile_position=(pb, 0))
                        rhs2 = bass.AP(tensor=T2.tensor,
                                       offset=pb * pitch2 + 2 * jp * 256 + 32 + dx,
                                       ap=[[pitch2, 64], [16, 2], [256, 2], [1, 224]])
                        nc.tensor.matmul(psX[:, :, :],
                                         W2b[pb:pb + 64, dx, :, :], rhs2,
                                         start=False, stop=(dx == 2),
                                         perf_mode=DR, tile_position=(pb, 0))
                    if jp % 2 != 0:
                        p2v = bass.AP(tensor=psX.tensor, offset=0,
                                      ap=[[448, 64], [32, 14], [2, 7], [16, 2], [1, 2]])
                        S2 = tmp.tile([64, 14, 7], f16, tag="s2", name="S2")
                        nc.vector.tensor_reduce(out=S2, in_=p2v,
                                                axis=mybir.AxisListType.XY,
                                                op=ALU.max)
                        S3 = tmp.tile([64, 14, 7], fp32, tag="s3", name="S3")
                        nc.vector.tensor_scalar(out=S3, in0=S2,
                                                scalar1=CV[0:64, 0:1],
                                                scalar2=CV[0:64, 1:2],
                                                op0=ALU.mult, op1=ALU.add)
                        S4 = tmp.tile([64, 14, 7], fp32, tag="s4", name="S4")
                        nc.vector.tensor_scalar(out=S4, in0=S3, scalar1=3.0,
                                                scalar2=float(C_RND),
                                                op0=ALU.min, op1=ALU.add)
                        nc.scalar.activation(
                            out=T3[pb:pb + 64, 2 * jp:2 * jp + 2, 0:7, 0:7],
                            in_=S4, func=AF.Relu, bias=CV[0:64, 4:5], scale=1.0)
                    else:
                        # path D: ACT drains PSUM with fused BN affine + relu
                        psr = bass.AP(tensor=psX.tensor, offset=0,
                                      ap=[[448, 64], [224, 2], [16, 14], [1, 14]])
                        E1 = tmp.tile([64, 2, 14, 14], f16, tag="e1", name="E1")
                        nc.scalar.activation(out=E1, in_=psr, func=AF.Relu,
                                             bias=CV[0:64, 1:2],
                                             scale=CV[0:64, 0:1])
                        e1v = E1.rearrange("p b (y2 dy) x -> p b y2 dy x", dy=2)
                        E2 = tmp.tile([64, 2, 7, 14], f16, tag="e2", name="E2")
                        nc.vector.tensor_max(E2, e1v[:, :, :, 0, :],
                                             e1v[:, :, :, 1, :])
                        e2v = E2.rearrange("p b y (x2 dx) -> p b y x2 dx", dx=2)
                        E3 = tmp.tile([64, 2, 7, 7], f16, tag="e3", name="E3")
                        nc.vector.tensor_max(E3, e2v[:, :, :, :, 0],
                                             e2v[:, :, :, :, 1])
                        S4 = tmp.tile([64, 2, 7, 7], fp32, tag="s4b", name="S4")
                        nc.vector.tensor_scalar(out=S4, in0=E3, scalar1=3.0,
                                                scalar2=float(C_RND),
                                                op0=ALU.min, op1=ALU.add)
                        nc.scalar.activation(
                            out=T3[pb:pb + 64, 2 * jp:2 * jp + 2, 0:7, 0:7],
                            in_=S4, func=AF.Relu, bias=CV[0:64, 4:5], scale=1.0)

            # ---- conv3 (fp8 DoubleRow, VALID) + bn2 + quant ----
            pitch3 = 33 * 128
            Q3v = Q3.rearrange("p (y x) b -> p b y x", y=5)
            for q2 in range(4):
                for si in range(2):
                    pb = 64 * si
                    ps3 = psC.tile([64, 8, 8, 5], fp32, tag="mc", name="ps3")
                    for dx in range(3):
                        rhs = bass.AP(tensor=T3.tensor,
                                      offset=pb * pitch3 + 8 * q2 * 128 + dx,
                                      ap=[[pitch3, 64], [16, 2], [16, 64], [1, 5]])
                        nc.tensor.matmul(ps3[:, :, :, :],
                                         W3[pb:pb + 64, dx, :, :], rhs,
                                         start=(dx == 0), stop=False,
                                         perf_mode=DR, tile_position=(pb, 0))
                        rhs2 = bass.AP(tensor=T3.tensor,
                                       offset=pb * pitch3 + 8 * q2 * 128 + 32 + dx,
                                       ap=[[pitch3, 64], [16, 2], [16, 64], [1, 5]])
                        nc.tensor.matmul(ps3[:, :, :, :],
                                         W3b[pb:pb + 64, dx, :, :], rhs2,
                                         start=False, stop=(dx == 2),
                                         perf_mode=DR, tile_position=(pb, 0))
                    U1 = tmp.tile([64, 8, 8, 5], fp32, tag="u1", name="U1")
                    nc.scalar.activation(out=U1, in_=ps3, func=AF.Identity,
                                         bias=CV[0:64, 3:4], scale=CV[0:64, 2:3])
                    U2 = tmp.tile([64, 8, 8, 5], fp32, tag="u2", name="U2")
                    nc.vector.tensor_scalar(out=U2, in0=U1, scalar1=3.0,
                                            scalar2=float(C_RND),
                                            op0=ALU.min, op1=ALU.add)
                    nc.scalar.activation(
                        out=Q3v[pb:pb + 64, 8 * q2:8 * q2 + 8, :, :],
                        in_=U2[:, :, 0:5, :], func=AF.Relu,
                        bias=CV[0:64, 4:5], scale=1.0)

            # ---- fc1 input staging: deferred 4 DMAs (jh x set), emitted
            # after the NEXT chunk's im2col so they can't head-of-line block
            # the SP queue's prefetch ----
            def _stage(Q3=Q3, img0=img0):
                for jh in range(2):
                    nk = 13 if jh == 0 else 12
                    for si in range(2):
                        src = bass.AP(tensor=Q3.tensor,
                                      offset=64 * si * 800 + 32 * jh,
                                      ap=[[800, 64], [64, nk], [1, 32]])
                        dst = bass.AP(tensor=F2.tensor,
                                      offset=64 * jh * (14 * 512) + img0 + 32 * si,
                                      ap=[[14 * 512, 64], [512, nk], [1, 32]])
                        nc.sync.dma_start(out=dst, in_=src)
            pending_fc.append(_stage)

        while pending_fc:
            pending_fc.pop(0)()

        # ---- fc1 (fp8 DoubleRow over 7 chunk-pairs) ----
        for m in range(4):
            psf = psB.tile([128, 512], fp32, tag="c2", name="psf")
            for u in range(7):
                rhs = bass.AP(tensor=F2.tensor, offset=2 * u * 512,
                              ap=[[14 * 512, 128], [512, 2], [1, 512]])
                nc.tensor.matmul(psf, FW1[:, m, u, :, :], rhs,
                                 start=(u == 0), stop=(u == 6), perf_mode=DR)
            h = sg.tile([128, 512], fp32, name=f"H1v_{m}")
            H1[m] = h
            nc.scalar.activation(out=h, in_=psf, func=AF.Identity,
                                 bias=FB1[:, m:m + 1], scale=c13)

        # ---- fc2 (fc1 acts stationary -> logits batch-on-partitions) ----
        for qq in range(4):
            psj = psC.tile([128, 10], fp32, tag="mc", name="psj")
            for k2 in range(4):
                nc.tensor.matmul(psj, H1[k2][:, 128 * qq:128 * (qq + 1)],
                                 FW2[:, k2, :], start=(k2 == 0), stop=(k2 == 3))
            v = tmp.tile([128, 10], fp32, tag="lg", name="v")
            nc.vector.tensor_add(v, psj, FB2T)
            mx = tmp.tile([128, 1], fp32, tag="mx", name="mx")
            nc.vector.reduce_max(out=mx, in_=v, axis=AX)
            tt = tmp.tile([128, 10], fp32, tag="tt", name="tt")
            nc.vector.tensor_scalar(out=tt, in0=v, scalar1=mx, scalar2=None,
                                    op0=ALU.subtract)
            ee = tmp.tile([128, 10], fp32, tag="ee", name="ee")
            ss = tmp.tile([128, 1], fp32, tag="ss", name="ss")
            nc.scalar.activation(out=ee, in_=tt, func=AF.Exp, accum_out=ss)
            ll = tmp.tile([128, 1], fp32, tag="ll", name="ll")
            nc.scalar.activation(out=ll, in_=ss, func=AF.Ln)
            oo = tmp.tile([128, 10], fp32, tag="oo", name="oo")
            nc.vector.tensor_scalar(out=oo, in0=tt, scalar1=ll, scalar2=None,
                                    op0=ALU.subtract)
            nc.sync.dma_start(out=d_out[128 * qq:128 * (qq + 1), :], in_=oo)

    nc.finalize()
    return nc


_NC_CACHE = None
TRACE = False
TRACE_DIR = None


def kernel(**inputs):
    global _NC_CACHE
    x = np.asarray(inputs["x"], dtype=np.float32).reshape(4096, 28, 28)
    consts = _prep(**{k: v for k, v in inputs.items() if k != "x"})

    if _NC_CACHE is None:
        _NC_CACHE = _build_nc()
    nc = _NC_CACHE

    in_maps = []
    for corei in range(N_CORES):
        hi, mid, lo = _split_x(x[corei * B_CORE:(corei + 1) * B_CORE])
        m = dict(s_hi=hi, s_mid=mid, s_lo=lo)
        m.update(consts)
        in_maps.append(m)

    from concourse.bass_utils import run_bass_kernel_spmd
    res = run_bass_kernel_spmd(nc, in_maps, core_ids=list(range(N_CORES)),
                               trace=TRACE, tmpdir=TRACE_DIR)
    if res.exec_time_ns is not None:
        print(f"HW exec time: {res.exec_time_ns} ns")
        print(f"mean exec time: {res.mean_exec_time_ns} ns")
    out = np.concatenate([r["out"] for r in res.results], axis=0)
    return out.astype(np.float32)


# revision 11
# speedup vs baseline: 1.0426x; 1.0252x over previous
"""Trainium2 Bass kernel for nn_Net_12481174962824 (binarized CNN) — v2.

Data-parallel over 8 cores (512 images each). Within a core, images are
processed in chunks of 64, split into two 32-image sets living on partition
halves (set A -> partitions 0-63, set B -> 64-127) so every post-conv
pool/quant chain runs at full 128-partition width with no cross-partition
copies.

  conv1: bf16 two-split (hi/mid) im2col matmul, K=20 per 32-row group
    (2 splits x 9 taps + 2 ones-rows carrying the bias 3*b1 as hi/lo bf16
    weights; the x3 activation scale is folded into the +/-3 weights).
    4 row-groups x 2 col-groups = 8-way PE tiling, PSUM [128, 14, 28].
  pool+quant: banks alternate between a DVE pooled-reduce path and an
    ACT-drain + DVE tensor_max path to balance the two PSUM-reader engines;
    round via min(3,x)+2^23 (RNE) then ACT Relu(x-C) -> fp8.
  conv2/conv3: fp8e4 DoubleRow matmuls — activations {0..3} and +/-1
    weights are exact in fp8; each MM's in-cell pair covers taps (dy,dy+1)
    via the 16-byte row pitch, so 3 taps/column need 2 MMs per dx (6 total
    per 9-tap conv). Sets run as separate row/col tile groups.
  fc1: fp8 DoubleRow over 14 feature-chunks (zero-padded), fc2 fp32 with
    fc1 activations stationary so logits land batch-on-partitions.
"""

import numpy as np
import ml_dtypes

BF16 = ml_dtypes.bfloat16
FP8 = ml_dtypes.float8_e4m3
F32 = np.float32
C_RND = np.float32(12582912.0)  # 1.5 * 2**23: (x + C) - C rounds to int (RNE)
N_CORES = 8
B_CORE = 512
NB = 64               # images per chunk
NCHUNK = B_CORE // NB
NBG = NB // 4         # images per conv1 row-group (16)


def _f32(x):
    return np.asarray(x, dtype=np.float32)


def _prep(w1, b1, w2, g1, be1, m1, v1, w3, g2, be2, m2, v2, fw1, fb1, fw2, fb2):
    """Host prep of small weight tensors. Returns dict of np arrays."""
    sg = lambda w: np.where(_f32(w) >= 0, np.float32(1), np.float32(-1))

    # conv1 lhsT [128, 64]: rows 32g + 9s + t = 3*sign(w1); rows 32g+18/19 = bias 3*b1 hi/lo
    w1b = sg(w1)  # [64,1,3,3]
    base = (3.0 * w1b[:, 0].reshape(64, 9).T).astype(F32)  # [9 taps, 64]
    b3 = (3.0 * _f32(b1)).astype(F32)
    b3hi = b3.astype(BF16)
    b3lo = (b3 - b3hi.astype(F32)).astype(BF16)
    w1l = np.zeros((128, 64), dtype=BF16)
    for g in range(4):
        for s in range(3):
            w1l[32 * g + 9 * s: 32 * g + 9 * s + 9, :] = base.astype(BF16)
        w1l[32 * g + 27, :] = b3hi
        w1l[32 * g + 28, :] = b3lo

    # conv2/conv3 DoubleRow weights: [128, 3(dx), 2(j), 64] with rows 64-127 duplicated
    def dr_pair(w):
        wb = sg(w)  # [cout, cin, dy, dx]
        a = np.zeros((64, 3, 2, 64), dtype=FP8)
        b = np.zeros((64, 3, 2, 64), dtype=FP8)
        for dx in range(3):
            a[:, dx, 0, :] = wb[:, :, 0, dx].T.astype(FP8)
            a[:, dx, 1, :] = wb[:, :, 1, dx].T.astype(FP8)
            b[:, dx, 0, :] = wb[:, :, 2, dx].T.astype(FP8)
        a2 = np.concatenate([a, a], axis=0).reshape(128, 3 * 2 * 64)
        b2 = np.concatenate([b, b], axis=0).reshape(128, 3 * 2 * 64)
        return a2, b2

    w2l, w2l2 = dr_pair(w2)
    w3l, w3l2 = dr_pair(w3)

    # BN folds (fp32 arithmetic like the reference); activations carry a 3x scale
    def fold(g, be, m, v):
        rs = (np.float32(1.0) / np.sqrt(_f32(v) + np.float32(1e-4))).astype(F32)
        inv = (_f32(g) * rs).astype(F32)
        assert (inv > 0).all(), "negative BN scale: pool/quant commute breaks"
        s = inv.astype(F32)                                   # 3*c*inv, c=1/3
        b = (np.float32(3.0) * (_f32(be) - _f32(m) * inv)).astype(F32)
        return s, b

    s1v, b1v = fold(g1, be1, m1, v1)
    s2v, b2v = fold(g2, be2, m2, v2)
    cvec = np.zeros((128, 6), dtype=F32)
    for half in range(2):
        cvec[64 * half:64 * half + 64, 0] = s1v
        cvec[64 * half:64 * half + 64, 1] = b1v
        cvec[64 * half:64 * half + 64, 2] = s2v
        cvec[64 * half:64 * half + 64, 3] = b2v
    cvec[:, 4] = -C_RND

    # fc1 DoubleRow lhsT [128, 4(m), 7(u), 2(j), 128]:
    # partition p = 64*jh + ch holds feature (4u+2j+jh)*64 + ch
    fw1b = sg(fw1)  # [512, 1600]
    fw1l = np.zeros((128, 4, 7, 2, 128), dtype=FP8)
    for m in range(4):
        for u in range(7):
            for j in range(2):
                for jh in range(2):
                    p = 4 * u + 2 * j + jh
                    if p >= 25:
                        continue
                    fw1l[64 * jh:64 * jh + 64, m, u, j, :] = (
                        fw1b[128 * m:128 * (m + 1), 64 * p:64 * (p + 1)].T.astype(FP8))
    fw1l = fw1l.reshape(128, 4 * 7 * 2 * 128)

    # fc2 lhsT [128, 4, 10] fp32: row j of chunk k2 = fc1-feature 128*k2+j
    fw2l = np.zeros((128, 4, 10), dtype=F32)
    for k2 in range(4):
        fw2l[:, k2, :] = _f32(fw2)[:, 128 * k2:128 * (k2 + 1)].T
    fw2l = fw2l.reshape(128, 40)

    fb1v = _f32(fb1).reshape(4, 128).T.copy()  # [128, 4]
    fb2v = _f32(fb2).reshape(10, 1).copy()     # [10, 1]
    onesv = np.ones(2 * 16 * 840, dtype=BF16)
    return dict(w1l=w1l, w2l=w2l, w2l2=w2l2, w3l=w3l, w3l2=w3l2, cvec=cvec,
                fw1l=fw1l, fw2l=fw2l, fb1v=fb1v, fb2v=fb2v, onesv=onesv)


def _split_x(x_shard):
    """[512,28,28] f32 -> three padded bf16 split tensors [512*900+64]."""
    S = np.zeros((B_CORE, 30, 30), dtype=F32)
    S[:, 1:29, 1:29] = x_shard
    S = S.reshape(-1)
    hi = S.astype(BF16)
    r = (S - hi.astype(F32)).astype(F32)
    mid = r.astype(BF16)
    lo = (r - mid.astype(F32)).astype(BF16)
    pad = np.zeros(64, dtype=BF16)
    return (np.concatenate([hi, pad]), np.concatenate([mid, pad]),
            np.concatenate([lo, pad]))


def _build_nc():
    import concourse.bass as bass
    import concourse.bacc as bacc
    import concourse.tile as tile
    import concourse.mybir as mybir
    from contextlib import ExitStack

    fp32 = mybir.dt.float32
    bf16 = mybir.dt.bfloat16
    f16 = mybir.dt.float16
    f8 = mybir.dt.float8e4
    AX = mybir.AxisListType.X
    AF = mybir.ActivationFunctionType
    ALU = mybir.AluOpType
    DR = mybir.MatmulPerfMode.DoubleRow
    c13 = float(np.float32(1.0) / np.float32(3.0))

    nc = bacc.Bacc("TRN2", target_bir_lowering=False)
    d_shi = nc.dram_tensor("s_hi", [B_CORE * 900 + 64], bf16, kind="ExternalInput")
    d_smid = nc.dram_tensor("s_mid", [B_CORE * 900 + 64], bf16, kind="ExternalInput")
    d_slo = nc.dram_tensor("s_lo", [B_CORE * 900 + 64], bf16, kind="ExternalInput")
    d_w1 = nc.dram_tensor("w1l", [128, 64], bf16, kind="ExternalInput")
    d_w2 = nc.dram_tensor("w2l", [128, 384], f8, kind="ExternalInput")
    d_w2b = nc.dram_tensor("w2l2", [128, 384], f8, kind="ExternalInput")
    d_w3 = nc.dram_tensor("w3l", [128, 384], f8, kind="ExternalInput")
    d_w3b = nc.dram_tensor("w3l2", [128, 384], f8, kind="ExternalInput")
    d_cv = nc.dram_tensor("cvec", [128, 6], fp32, kind="ExternalInput")
    d_fw1 = nc.dram_tensor("fw1l", [128, 4 * 7 * 2 * 128], f8, kind="ExternalInput")
    d_fw2 = nc.dram_tensor("fw2l", [128, 40], fp32, kind="ExternalInput")
    d_fb1 = nc.dram_tensor("fb1v", [128, 4], fp32, kind="ExternalInput")
    d_fb2 = nc.dram_tensor("fb2v", [10, 1], fp32, kind="ExternalInput")
    d_ones = nc.dram_tensor("onesv", [2 * 16 * 840], bf16, kind="ExternalInput")
    d_out = nc.dram_tensor("out", [B_CORE, 10], fp32, kind="ExternalOutput")

    splits = [d_shi, d_smid, d_slo]

    with tile.TileContext(nc) as tc, ExitStack() as ctx:
        sg = ctx.enter_context(tc.tile_pool(name="sg", bufs=1))
        tmp = ctx.enter_context(tc.tile_pool(name="tmp", bufs=6))
        psA = ctx.enter_context(tc.tile_pool(name="psA", bufs=3, space="PSUM"))
        psB = ctx.enter_context(tc.tile_pool(name="psB", bufs=3, space="PSUM"))
        psC = ctx.enter_context(tc.tile_pool(name="psC", bufs=2, space="PSUM"))

        # --- weights / constants ---
        W1 = sg.tile([128, 64], bf16)
        nc.sync.dma_start(out=W1, in_=d_w1[:, :])
        W2 = sg.tile([128, 3, 2, 64], f8)
        nc.sync.dma_start(out=W2, in_=d_w2[:, :].rearrange("p (d j c) -> p d j c", d=3, j=2))
        W2b = sg.tile([128, 3, 2, 64], f8)
        nc.sync.dma_start(out=W2b, in_=d_w2b[:, :].rearrange("p (d j c) -> p d j c", d=3, j=2))
        W3 = sg.tile([128, 3, 2, 64], f8)
        nc.sync.dma_start(out=W3, in_=d_w3[:, :].rearrange("p (d j c) -> p d j c", d=3, j=2))
        W3b = sg.tile([128, 3, 2, 64], f8)
        nc.sync.dma_start(out=W3b, in_=d_w3b[:, :].rearrange("p (d j c) -> p d j c", d=3, j=2))
        CV = sg.tile([128, 6], fp32)
        nc.sync.dma_start(out=CV, in_=d_cv[:, :])
        FW1 = sg.tile([128, 4, 7, 2, 128], f8)
        FW2 = sg.tile([128, 4, 10], fp32)
        FB1 = sg.tile([128, 4], fp32)
        FB2T = sg.tile([128, 10], fp32)

        # --- persistent ping-pong activation tiles ---
        T1s = [sg.tile([128, 16, 840], bf16, name=f"T1_{i}") for i in range(2)]
        T2s = [sg.tile([128, 33, 16, 16], f8, name=f"T2_{i}") for i in range(2)]
        T3s = [sg.tile([128, 33, 8, 16], f8, name=f"T3_{i}") for i in range(2)]
        Q3s = [sg.tile([128, 25, 32], f8, name=f"Q3_{i}") for i in range(2)]
        F2 = sg.tile([128, 14, 512], f8)
        H1 = [None] * 4

        # one-time pads/ones
        for i in range(2):
            T2, T3 = T2s[i], T3s[i]
            nc.gpsimd.memset(T2[:, :, 0, :], 0)
            nc.gpsimd.memset(T2[:, :, 15, :], 0)
            nc.gpsimd.memset(T2[:, :, 1:15, 0], 0)
            nc.gpsimd.memset(T2[:, :, 1:15, 15], 0)
            nc.gpsimd.memset(T2[:, 32, 1:15, 1:15], 0)
            nc.gpsimd.memset(T3[:, :, 7, :], 0)
            nc.gpsimd.memset(T3[:, :, 0:7, 7:16], 0)
            nc.gpsimd.memset(T3[:, 32, :, :], 0)
            if i == 0:
                for g in range(4):
                    nc.sync.dma_start(
                        out=T1s[i][32 * g + 27:32 * g + 29, :, :].rearrange("p b x -> p (b x)"),
                        in_=bass.AP(tensor=d_ones[:].tensor, offset=0,
                                    ap=[[13440, 2], [1, 13440]]))
        nc.vector.memset(F2[64:128, 12, :], 0)
        nc.vector.memset(F2[:, 13, :], 0)
        # touch Exp/Ln act tables now so the loads don't stall the softmax tail
        warm = sg.tile([128, 2], fp32, name="warm")
        nc.gpsimd.memset(warm, 1.0)
        nc.scalar.activation(out=warm[:, 0:1], in_=warm[:, 0:1], func=AF.Exp)
        nc.scalar.activation(out=warm[:, 1:2], in_=warm[:, 1:2], func=AF.Ln)

        pending_fc = []
        for ch in range(NCHUNK):
            img0 = ch * NB
            T1, T2, T3, Q3 = T1s[ch % 2], T2s[ch % 2], T3s[ch % 2], Q3s[ch % 2]

            # ---- conv1 im2col DMAs: one per (group, split, dy) ----
            for g in range(4):
                for s in range(3):
                    for dy in range(3):
                        src = bass.AP(
                            tensor=splits[s][:].tensor,
                            offset=(img0 + 16 * g) * 900 + 30 * dy,
                            ap=[[1, 3], [900, 16], [1, 840]])
                        r0 = 32 * g + 9 * s + 3 * dy
                        # lo split goes via the Pool engine's SWDGE queue so
                        # the SP/HWDGE path only carries 24 DMAs per chunk
                        if s == 2:
                            nc.gpsimd.dma_start(out=T1[r0:r0 + 3, :, :], in_=src)
                        else:
                            nc.sync.dma_start(out=T1[r0:r0 + 3, :, :], in_=src)
            while pending_fc:
                pending_fc.pop(0)()
            if ch == 0:
                # deferred loads: not needed until chunk 1 / fc, so they sit
                # behind chunk 0's im2col prefetch in the SP queue
                for g in range(4):
                    nc.sync.dma_start(
                        out=T1s[1][32 * g + 27:32 * g + 29, :, :].rearrange("p b x -> p (b x)"),
                        in_=bass.AP(tensor=d_ones[:].tensor, offset=0,
                                    ap=[[13440, 2], [1, 13440]]))
                nc.sync.dma_start(out=FW1, in_=d_fw1[:, :].rearrange(
                    "p (m u j c) -> p m u j c", m=4, u=7, j=2))
                nc.sync.dma_start(out=FW2, in_=d_fw2[:, :].rearrange(
                    "p (k m) -> p k m", k=4))
                nc.sync.dma_start(out=FB1, in_=d_fb1[:, :])
                fb2b = bass.AP(tensor=d_fb2[:, :].tensor, offset=0,
                               ap=[[0, 128], [1, 10]])
                nc.sync.dma_start(out=FB2T, in_=fb2b)

            T1v = T1.rearrange("p b (h y x) -> p b h y x", h=2, y=14, x=30)

            # ---- conv1 + pool + quant ----
            for bb in range(16):
                for half in range(2):
                    ys = 1 + 7 * half
                    P0 = psA.tile([128, 14, 28], fp32, tag="c1", name="P0")
                    P1 = psA.tile([128, 14, 28], fp32, tag="c1", name="P1")
                    for g in range(4):
                        P = (P0, P1)[g % 2]
                        colg = 64 * (g // 2)
                        nc.tensor.matmul(
                            P[colg:colg + 64, :, :],
                            W1[32 * g:32 * g + 29, :],
                            T1v[32 * g:32 * g + 29, bb, half, :, 0:28],
                            start=True, stop=True, skip_group_check=True,
                            tile_position=(32 * g, colg))
                    R2P = tmp.tile([128, 2, 7, 14], fp32, tag="r2p", name="R2P")
                    if bb % 16 not in (0, 2, 4, 6, 8, 10, 12, 14, 13, 5):
                        # path R: DVE pooled-reduce straight from PSUM
                        for pi, P in enumerate((P0, P1)):
                            psv = P.rearrange(
                                "p (y2 dy) (x2 dx) -> p y2 x2 dy dx",
                                dy=2, dx=2)
                            nc.vector.tensor_reduce(out=R2P[:, pi, :, :],
                                                    in_=psv,
                                                    axis=mybir.AxisListType.XY,
                                                    op=ALU.max)
                    else:
                        # path D: ACT drains PSUM so DVE only does the
                        # cheap SBUF max pair + round (drain-load balance)
                        for pi, P in enumerate((P0, P1)):
                            D1 = tmp.tile([128, 14, 28], fp32, tag="d1", name="D1")
                            nc.scalar.activation(out=D1, in_=P, func=AF.Copy,
                                                 bias=0.0, scale=1.0)
                            d1v = D1.rearrange("p (y2 dy) x -> p y2 dy x", dy=2)
                            D2 = tmp.tile([128, 7, 28], fp32, tag="d2", name="D2")
                            nc.vector.tensor_max(D2, d1v[:, :, 0, :],
                                                 d1v[:, :, 1, :])
                            d2v = D2.rearrange("p y (x2 dx) -> p y x2 dx", dx=2)
                            nc.vector.tensor_max(R2P[:, pi, :, :],
                                                 d2v[:, :, :, 0],
                                                 d2v[:, :, :, 1])
                    # shared round + single paired write into slots bb, bb+16
                    R3P = tmp.tile([128, 2, 7, 14], fp32, tag="r3p", name="R3P")
                    nc.vector.tensor_scalar(out=R3P, in0=R2P, scalar1=3.0,
                                            scalar2=float(C_RND),
                                            op0=ALU.min, op1=ALU.add)
                    T2p = T2[:, 0:32, :, :].rearrange(
                        "p (b2 sl) y x -> p sl b2 y x", b2=2)
                    nc.scalar.activation(out=T2p[:, bb, :, ys:ys + 7, 1:15],
                                         in_=R3P, func=AF.Relu,
                                         bias=CV[:, 4:5], scale=1.0)

            # ---- conv2 (fp8 DoubleRow, 6 MMs per set, per-set dst banks) ----
            pitch2 = 33 * 256
            for jp in range(16):
                for si in range(2):
                    pb = 64 * si
                    psX = psB.tile([64, 2, 224], fp32, tag="c2", name="psX")
                    for dx in range(3):
                        rhs = bass.AP(tensor=T2.tensor,
                                      offset=pb * pitch2 + 2 * jp * 256 + dx,
                                      ap=[[pitch2, 64], [16, 2], [256, 2], [1, 224]])
                        nc.tensor.matmul(psX[:, :, :],
                                         W2[pb:pb + 64, dx, :, :], rhs,
                                         start=(dx == 0), stop=False,
                                         perf_mode=DR, tile_position=(pb, 0))
                        rhs2 = bass.AP(tensor=T2.tensor,
                                       offset=pb * pitch2 + 2 * jp * 256 + 32 + dx,
                                       ap=[[pitch2, 64], [16, 2], [256, 2], [1, 224]])
                        nc.tensor.matmul(psX[:, :, :],
                                         W2b[pb:pb + 64, dx, :, :], rhs2,
                                         start=False, stop=(dx == 2),
                                         perf_mode=DR, tile_position=(pb, 0))
                    if jp % 2 != 0:
                        p2v = bass.AP(tensor=psX.tensor, offset=0,
                                      ap=[[448, 64], [32, 14], [2, 7], [16, 2], [1, 2]])
                        S2 = tmp.tile([64, 14, 7], f16, tag="s2", name="S2")
                        nc.vector.tensor_reduce(out=S2, in_=p2v,
                                                axis=mybir.AxisListType.XY,
                                                op=ALU.max)
                        S3 = tmp.tile([64, 14, 7], fp32, tag="s3", name="S3")
                        nc.vector.tensor_scalar(out=S3, in0=S2,
                                                scalar1=CV[0:64, 0:1],
                                                scalar2=CV[0:64, 1:2],
                                                op0=ALU.mult, op1=ALU.add)
                        S4 = tmp.tile([64, 14, 7], fp32, tag="s4", name="S4")
                        nc.vector.tensor_scalar(out=S4, in0=S3, scalar1=3.0,
                                                scalar2=float(C_RND),
                                                op0=ALU.min, op1=ALU.add)
                        nc.scalar.activation(
                            out=T3[pb:pb + 64, 2 * jp:2 * jp + 2, 0:7, 0:7],
                            in_=S4, func=AF.Relu, bias=CV[0:64, 4:5], scale=1.0)
                    else:
                        # path D: ACT drains PSUM with fused BN affine + relu
                        psr = bass.AP(tensor=psX.tensor, offset=0,
                                      ap=[[448, 64], [224, 2], [16, 14], [1, 14]])
                        E1 = tmp.tile([64, 2, 14, 14], f16, tag="e1", name="E1")
                        nc.scalar.activation(out=E1, in_=psr, func=AF.Relu,
                                             bias=CV[0:64, 1:2],
                                             scale=CV[0:64, 0:1])
                        e1v = E1.rearrange("p b (y2 dy) x -> p b y2 dy x", dy=2)
                        E2 = tmp.tile([64, 2, 7, 14], f16, tag="e2", name="E2")
                        nc.vector.tensor_max(E2, e1v[:, :, :, 0, :],
                                             e1v[:, :, :, 1, :])
                        e2v = E2.rearrange("p b y (x2 dx) -> p b y x2 dx", dx=2)
                        E3 = tmp.tile([64, 2, 7, 7], f16, tag="e3", name="E3")
                        nc.vector.tensor_max(E3, e2v[:, :, :, :, 0],
                                             e2v[:, :, :, :, 1])
                        S4 = tmp.tile([64, 2, 7, 7], fp32, tag="s4b", name="S4")
                        nc.vector.tensor_scalar(out=S4, in0=E3, scalar1=3.0,
                                                scalar2=float(C_RND),
                                                op0=ALU.min, op1=ALU.add)
                        nc.scalar.activation(
                            out=T3[pb:pb + 64, 2 * jp:2 * jp + 2, 0:7, 0:7],
                            in_=S4, func=AF.Relu, bias=CV[0:64, 4:5], scale=1.0)

            # ---- conv3 (fp8 DoubleRow, VALID) + bn2 + quant ----
            pitch3 = 33 * 128
            Q3v = Q3.rearrange("p (y x) b -> p b y x", y=5)
            for q2 in range(4):
                for si in range(2):
                    pb = 64 * si
                    ps3 = psC.tile([64, 8, 8, 5], fp32, tag="mc", name="ps3")
                    for dx in range(3):
                        rhs = bass.AP(tensor=T3.tensor,
                                      offset=pb * pitch3 + 8 * q2 * 128 + dx,
                                      ap=[[pitch3, 64], [16, 2], [16, 64], [1, 5]])
                        nc.tensor.matmul(ps3[:, :, :, :],
                                         W3[pb:pb + 64, dx, :, :], rhs,
                                         start=(dx == 0), stop=False,
                                         perf_mode=DR, tile_position=(pb, 0))
                        rhs2 = bass.AP(tensor=T3.tensor,
                                       offset=pb * pitch3 + 8 * q2 * 128 + 32 + dx,
                                       ap=[[pitch3, 64], [16, 2], [16, 64], [1, 5]])
                        nc.tensor.matmul(ps3[:, :, :, :],
                                         W3b[pb:pb + 64, dx, :, :], rhs2,
                                         start=False, stop=(dx == 2),
                                         perf_mode=DR, tile_position=(pb, 0))
                    U1 = tmp.tile([64, 8, 8, 5], fp32, tag="u1", name="U1")
                    nc.scalar.activation(out=U1, in_=ps3, func=AF.Identity,
                                         bias=CV[0:64, 3:4], scale=CV[0:64, 2:3])
                    U2 = tmp.tile([64, 8, 8, 5], fp32, tag="u2", name="U2")
                    nc.vector.tensor_scalar(out=U2, in0=U1, scalar1=3.0,
                                            scalar2=float(C_RND),
                                            op0=ALU.min, op1=ALU.add)
                    nc.scalar.activation(
                        out=Q3v[pb:pb + 64, 8 * q2:8 * q2 + 8, :, :],
                        in_=U2[:, :, 0:5, :], func=AF.Relu,
                        bias=CV[0:64, 4:5], scale=1.0)

            # ---- fc1 input staging: deferred 4 DMAs (jh x set), emitted
            # after the NEXT chunk's im2col so they can't head-of-line block
            # the SP queue's prefetch ----
            def _stage(Q3=Q3, img0=img0):
                for jh in range(2):
                    nk = 13 if jh == 0 else 12
                    for si in range(2):
                        src = bass.AP(tensor=Q3.tensor,
                                      offset=64 * si * 800 + 32 * jh,
                                      ap=[[800, 64], [64, nk], [1, 32]])
                        dst = bass.AP(tensor=F2.tensor,
                                      offset=64 * jh * (14 * 512) + img0 + 32 * si,
                                      ap=[[14 * 512, 64], [512, nk], [1, 32]])
                        nc.sync.dma_start(out=dst, in_=src)
            pending_fc.append(_stage)

        while pending_fc:
            pending_fc.pop(0)()

        # ---- fc1 (fp8 DoubleRow over 7 chunk-pairs) ----
        for m in range(4):
            psf = psB.tile([128, 512], fp32, tag="c2", name="psf")
            for u in range(7):
                rhs = bass.AP(tensor=F2.tensor, offset=2 * u * 512,
                              ap=[[14 * 512, 128], [512, 2], [1, 512]])
                nc.tensor.matmul(psf, FW1[:, m, u, :, :], rhs,
                                 start=(u == 0), stop=(u == 6), perf_mode=DR)
            h = sg.tile([128, 512], fp32, name=f"H1v_{m}")
            H1[m] = h
            nc.scalar.activation(out=h, in_=psf, func=AF.Identity,
                                 bias=FB1[:, m:m + 1], scale=c13)

        # ---- fc2 (fc1 acts stationary -> logits batch-on-partitions) ----
        for qq in range(4):
            psj = psC.tile([128, 10], fp32, tag="mc", name="psj")
            for k2 in range(4):
                nc.tensor.matmul(psj, H1[k2][:, 128 * qq:128 * (qq + 1)],
                                 FW2[:, k2, :], start=(k2 == 0), stop=(k2 == 3))
            v = tmp.tile([128, 10], fp32, tag="lg", name="v")
            nc.vector.tensor_add(v, psj, FB2T)
            mx = tmp.tile([128, 1], fp32, tag="mx", name="mx")
            nc.vector.reduce_max(out=mx, in_=v, axis=AX)
            tt = tmp.tile([128, 10], fp32, tag="tt", name="tt")
            nc.vector.tensor_scalar(out=tt, in0=v, scalar1=mx, scalar2=None,
                                    op0=ALU.subtract)
            ee = tmp.tile([128, 10], fp32, tag="ee", name="ee")
            ss = tmp.tile([128, 1], fp32, tag="ss", name="ss")
            nc.scalar.activation(out=ee, in_=tt, func=AF.Exp, accum_out=ss)
            ll = tmp.tile([128, 1], fp32, tag="ll", name="ll")
            nc.scalar.activation(out=ll, in_=ss, func=AF.Ln)
            oo = tmp.tile([128, 10], fp32, tag="oo", name="oo")
            nc.vector.tensor_scalar(out=oo, in0=tt, scalar1=ll, scalar2=None,
                                    op0=ALU.subtract)
            nc.sync.dma_start(out=d_out[128 * qq:128 * (qq + 1), :], in_=oo)

    nc.finalize()
    return nc


_NC_CACHE = None
TRACE = False
TRACE_DIR = None


def kernel(**inputs):
    global _NC_CACHE
    x = np.asarray(inputs["x"], dtype=np.float32).reshape(4096, 28, 28)
    consts = _prep(**{k: v for k, v in inputs.items() if k != "x"})

    if _NC_CACHE is None:
        _NC_CACHE = _build_nc()
    nc = _NC_CACHE

    in_maps = []
    for corei in range(N_CORES):
        hi, mid, lo = _split_x(x[corei * B_CORE:(corei + 1) * B_CORE])
        m = dict(s_hi=hi, s_mid=mid, s_lo=lo)
        m.update(consts)
        in_maps.append(m)

    from concourse.bass_utils import run_bass_kernel_spmd
    res = run_bass_kernel_spmd(nc, in_maps, core_ids=list(range(N_CORES)),
                               trace=TRACE, tmpdir=TRACE_DIR)
    if res.exec_time_ns is not None:
        print(f"HW exec time: {res.exec_time_ns} ns")
        print(f"mean exec time: {res.mean_exec_time_ns} ns")
    out = np.concatenate([r["out"] for r in res.results], axis=0)
    return out.astype(np.float32)
